# revision 36
# baseline (speedup 1.0000x reference)
"""Trainium2 Bass kernel for nn_AdaptiveBlock (dense_mlp).

Reference computation:
    y    = mean(x, axis=(2, 3))                   # (B, C) global avg pool
    h    = gelu(y @ W1)                           # (B, HID), exact erf gelu
    yp   = gelu(h @ W2)                           # (B, C)
    A    = yp @ WA + bA                           # (B, H)
    Bv   = yp @ WB + bB                           # (B, W)
    attn = sigmoid(A[:,None,:,None] * Bv[:,None,None,:])   # (B, 1, H, W)
    out  = broadcast(attn, (B, C, H, W))

Sharding: data-parallel over batch across 8 NeuronCores (4 batches/core),
weights replicated, no collectives.  Each core streams its 51.4 MB x-shard
through SBUF, row-reduces it, runs the tiny MLP on-chip, and writes only
its (56, 4*56) attention map.  The channel broadcast (and the h<->b axis
swap) is done on the host; it carries no information.

Key design points (measured on HW, see git history of this file):
  * x streams CHUNK-major: each 4-block tile completes one contraction
    chunk of mm1, so mm1 runs DURING the stream.  mm1 is computed
    TRANSPOSED (lhsT = W1 chunk [128c x 128hid], rhs = ysumT [128c x 4b]),
    which also yields h^T directly -- no PE transposes of h before mm2.
  * The x DMA issue is gated to <=3 outstanding tiles.  A deep HWDGE ring
    makes SDMA engine 15 run ~20% slow (starving every tile's completion
    semaphore); a shallow ring keeps the engines balanced.
  * Each tile's reduce is split DVE (blocks 0-1) / ACT accum_out (blocks
    2-3), halving per-tile reduce latency so the DMA gate always clears
    early and the stream never stalls.
  * Weights ride the SCALAR HWDGE ring (second hardware ring, full rate,
    no SWDGE interleave penalty); host pre-packs them into SBUF layout so
    each weight DMA is one big contiguous descriptor per partition.
  * Tail tiles shrink to 2/1/half blocks so the last reduce is ~1.7us.
  * Same-engine back-to-back RAW through SBUF is NOT interlocked (relaxed
    ordering): every such hazard carries an explicit self-wait.
  * Single sigmoid, single contiguous 50KB output DMA ([H, BS*W] layout).
"""

import numpy as np

import concourse.bass as bass
from concourse import mybir
from concourse.bass_utils import run_bass_kernel_spmd

B, C, HID, H, W = 32, 1024, 512, 56, 56
NCORES = 8
BS = B // NCORES          # 4 batches per core
ROWS = BS * C             # 4096 (b, c) rows per core
HW = H * W                # 3136
NCC = C // 128            # 8 contraction chunks for mm1
NQH = HID // 128          # 4 contraction chunks for mm2
NSLOT = 3                 # x slots (4 blocks each)
F32 = mybir.dt.float32
BF16 = mybir.dt.bfloat16

# x tiles, chunk-major: (cc, b_lo, b_hi, m_lo, m_hi).
# T0..T6 = chunks 0-6 (4 blocks); chunk 7 tapers: 2 + 1 + half + half.
TILES = [(cc, 0, 4, 0, HW) for cc in range(7)] + [
    (7, 0, 2, 0, HW),
    (7, 2, 3, 0, HW),
    (7, 3, 4, 0, HW // 2),
    (7, 3, 4, HW // 2, HW),
]
NT = len(TILES)  # 11

# per-tile reduce split: DVE takes the first ceil(nb/2) blocks, ACT the
# rest -- except the half-block tiles T9 (ACT) / T10 (DVE).
# cumulative (red_e, red_o) counts after tile m's reduce is done:
RE, RO = [], []
e = o = 0
for m, (cc, b0, b1, m0, m1) in enumerate(TILES):
    nb = b1 - b0
    if m1 - m0 < HW:          # half tiles: T9 on ACT, T10 on DVE
        if m0 == 0:
            o += 1
        else:
            e += 1
    else:
        e += 1                # DVE: one reduce over its block span
        o += nb - (nb + 1) // 2   # ACT: one accum per block
    RE.append(e)
    RO.append(o)


def build_bass(gelu_fn=None, debug_taps=False) -> bass.Bass:
    if gelu_fn is None:
        gelu_fn = mybir.ActivationFunctionType.Gelu
    sig_fn = mybir.ActivationFunctionType.Sigmoid
    copy_fn = mybir.ActivationFunctionType.Copy
    nc = bass.Bass()

    x_t = nc.dram_tensor("x", [ROWS, HW], F32, kind="ExternalInput")
    w1_t = nc.dram_tensor("W1p", [128, NCC * HID], BF16, kind="ExternalInput")
    w2_t = nc.dram_tensor("W2p", [128, NQH * C], BF16, kind="ExternalInput")
    wab_t = nc.dram_tensor(
        "WABp", [128, NCC * (H + W)], BF16, kind="ExternalInput"
    )
    ba_t = nc.dram_tensor("bAbf", [H], BF16, kind="ExternalInput")
    bb_t = nc.dram_tensor("bBbf", [W], BF16, kind="ExternalInput")
    # [h, (b w)] layout; host swaps to (b, h, w)
    out_t = nc.dram_tensor("out", [H, BS * W], F32, kind="ExternalOutput")
    dbg = {}
    if debug_taps:
        for name, shape in [
            ("dbg_ysum", [128, 33]), ("dbg_ysum_bf", [128, 32]),
            ("dbg_hT", [128, 2 * 2 * BS]), ("dbg_yp", [BS, C]),
            ("dbg_ypT", [128, NCC * BS]), ("dbg_ab", [BS, H + W]),
            ("dbg_bdiag", [BS, BS * W]),
        ]:
            dbg[name] = nc.dram_tensor(name, shape, F32, kind="ExternalOutput")

    # x row r = b*C + cc*128 + p; chunk-major view [cc, p, b, m]
    x_r = x_t[:, :].rearrange("(b q p) m -> q p b m", b=BS, q=NCC, p=128)

    # ---- SBUF ----
    x_sb = nc.alloc_sbuf_tensor("x_sb", [128, NSLOT, BS, HW], F32)
    ysum_sb = nc.alloc_sbuf_tensor("ysum_sb", [128, 33], F32)  # col32 scratch
    ysum_bf = nc.alloc_sbuf_tensor("ysum_bf", [128, 32], BF16)
    w1_sb = nc.alloc_sbuf_tensor("w1_sb", [128, NCC, HID], BF16)
    w2_sb = nc.alloc_sbuf_tensor("w2_sb", [128, NQH, C], BF16)
    wab_sb = nc.alloc_sbuf_tensor("wab_sb", [128, NCC, H + W], BF16)
    bab_sb = nc.alloc_sbuf_tensor("bab_sb", [1, H + W], BF16)
    ident_sb = nc.alloc_sbuf_tensor("ident_sb", [128, 128], BF16)
    ones_sb = nc.alloc_sbuf_tensor("ones_sb", [1, BS], BF16)
    mask_sb = nc.alloc_sbuf_tensor("mask_sb", [BS, BS, W], BF16)
    # hT layout [p, parity, gg, b]: hid group g = 2*gg + parity, so the
    # even/odd-bank gelus each write one contiguous slice
    hT_sb = nc.alloc_sbuf_tensor("hT_sb", [128, 2, 2, BS], BF16)
    yp_sb = nc.alloc_sbuf_tensor("yp_sb", [BS, C], BF16)
    ypT_sb = nc.alloc_sbuf_tensor("ypT_sb", [128, NCC, BS], BF16)
    ab_sb = nc.alloc_sbuf_tensor("ab_sb", [BS, H + W], BF16)
    bdiag_sb = nc.alloc_sbuf_tensor("bdiag_sb", [BS, BS, W], BF16)
    attn_sb = nc.alloc_sbuf_tensor("attn_sb", [H, BS, W], F32)
    scr_sb = nc.alloc_sbuf_tensor("scr_sb", [1, 1], F32)
    # dump target for the ACT-side reduces (activation must write a full
    # output even when only accum_out is wanted)
    red_scr = nc.alloc_sbuf_tensor("red_scr", [128, HW], BF16)

    # ---- PSUM (8 banks) ----
    # hT accumulators: even groups (0,2) / odd groups (1,3) in separate
    # banks so a gelu read never overlaps a PE write to the same bank.
    ps_hte = nc.alloc_psum_tensor("ps_hte", [128, 2, BS], F32)
    ps_hto = nc.alloc_psum_tensor("ps_hto", [128, 2, BS], F32)
    ps_yp1 = nc.alloc_psum_tensor("ps_yp1", [BS, C // 2], F32)
    ps_yp2 = nc.alloc_psum_tensor("ps_yp2", [BS, C // 2], F32)
    tp_a = nc.alloc_psum_tensor("tp_a", [128, 2, BS], BF16)
    tp_b = nc.alloc_psum_tensor("tp_b", [128, 2, BS], BF16)
    ps_ab = nc.alloc_psum_tensor("ps_ab", [BS, H + W], F32)
    ps_at = nc.alloc_psum_tensor("ps_at", [H, BS, W], F32)

    # ---- semaphores ----
    xdma_sems = [nc.alloc_semaphore(f"xdma_sem{n}") for n in range(NT)]
    w_sems = [nc.alloc_semaphore(f"w_sem{i}") for i in range(5)]
    id_sem = nc.alloc_semaphore("id_sem")
    ones_sem = nc.alloc_semaphore("ones_sem")
    red_e = nc.alloc_semaphore("red_e")        # DVE reduce progress
    red_o = nc.alloc_semaphore("red_o")        # ACT reduce progress
    add_sem = nc.alloc_semaphore("add_sem")    # last half-block folded in
    cast_sem = nc.alloc_semaphore("cast_sem")  # +1 per chunk cast (ACT)
    pe_sem = nc.alloc_semaphore("pe_sem")
    act_sem = nc.alloc_semaphore("act_sem")
    dve_sem = nc.alloc_semaphore("dve_sem")
    out_sem = nc.alloc_semaphore("out_sem")

    def red_wait(eng, m):
        """Wait until tile m's reduce is complete on both engines."""
        eng.wait_ge(red_e, RE[m])
        eng.wait_ge(red_o, RO[m])

    # PE ticks (pe_sem after every real PE op):
    #   1..28  mm1T chunks 0..6 (4 pairs each)
    #   29..32 mm1T chunk 7, groups g=0..3
    #   33..36 mm2 half0 q=0..3     37..40 mm2 half1 q=0..3
    #   41..44 tr0..tr3   45,46 m0,m1   47,48 tr4,tr5   49,50 m2,m3
    #   51,52 tr6,tr7   53,54 m4,m5   55,56 m6,m7   57 bias   58 outer
    # ACT increments (act_sem): gelu_hT_even 1, gelu_hT_odd 2,
    #   gelu_yp1 3, gelu_yp2 4, sigmoid 5
    # DVE increments (dve_sem): ypT copies 1..4, ab copy 5, bdiag 6

    with nc.Block() as blk:

        @blk.sync
        def _(sync):
            for n, (cc, b0, b1, m0, m1) in enumerate(TILES):
                if n >= NSLOT:
                    red_wait(sync, n - NSLOT)
                sync.dma_start(
                    out=x_sb[:, n % NSLOT, 0 : b1 - b0, 0 : m1 - m0],
                    in_=x_r[cc, :, b0:b1, m0:m1],
                ).then_inc(xdma_sems[n], 16)
            sync.wait_ge(act_sem, 5)
            sync.dma_start(
                out=out_t[:, :],
                in_=attn_sb[:, :, :].rearrange("h b w -> h (b w)"),
            ).then_inc(out_sem, 16)
            sync.wait_ge(out_sem, 16)

        @blk.vector
        def _(vec):
            vec.memset(ones_sb[:, :], 1.0).then_inc(ones_sem, 1)
            for n, (cc, b0, b1, m0, m1) in enumerate(TILES):
                nb = b1 - b0
                if m1 - m0 < HW:
                    if m0 == 0:
                        continue          # T9 is ACT's
                    vec.wait_ge(xdma_sems[n], 16)
                    vec.reduce_sum(       # T10 second half -> scratch col
                        out=ysum_sb[:, 32:33],
                        in_=x_sb[:, n % NSLOT, 0:1, 0 : m1 - m0],
                        axis=mybir.AxisListType.X,
                    ).then_inc(red_e, 1)
                    continue
                ne = (nb + 1) // 2        # DVE's share: first ne blocks
                vec.wait_ge(xdma_sems[n], 16)
                vec.reduce_sum(
                    out=ysum_sb[:, cc * BS + b0 : cc * BS + b0 + ne],
                    in_=x_sb[:, n % NSLOT, 0:ne, :],
                    axis=mybir.AxisListType.X,
                ).then_inc(red_e, 1)
            # fold the halves: col31 (ACT, T9) + col32 (own T10).
            # self-wait: same-engine RAW through SBUF is not interlocked
            vec.wait_ge(red_e, RE[NT - 1])
            vec.wait_ge(red_o, RO[NT - 1])
            nc.vector.tensor_add(
                out=ysum_sb[:, 31:32],
                in0=ysum_sb[:, 31:32],
                in1=ysum_sb[:, 32:33],
            ).then_inc(add_sem, 1)
            # ypT copies: tp_a{tr0,tr1}, tp_b{tr2,tr3}, tp_a{tr4,tr5}, ...
            for i, (bank, tick) in enumerate(
                [(tp_a, 42), (tp_b, 44), (tp_a, 48), (tp_b, 52)]
            ):
                vec.wait_ge(pe_sem, tick)
                nc.vector.tensor_copy(
                    out=ypT_sb[:, 2 * i : 2 * i + 2, :], in_=bank[:, :, :]
                ).then_inc(dve_sem, 1)
            vec.wait_ge(pe_sem, 57)
            nc.vector.tensor_copy(
                out=ab_sb[:, :], in_=ps_ab[:, :]
            ).then_inc(dve_sem, 1)
            # self-wait: the bdiag mul reads ab_sb the copy just wrote
            vec.wait_ge(dve_sem, 5)
            vec.wait_ge(id_sem, 4)
            # bdiag[b, bb, w] = Bv[b, w] * (b == bb)
            b_sl = ab_sb[:, H : H + W]
            b_bc = bass.AP(
                tensor=b_sl.tensor, offset=b_sl.offset,
                ap=[b_sl.ap[0], [0, BS], [b_sl.ap[1][0], W]],
            )
            nc.vector.tensor_mul(
                out=bdiag_sb[:, :, :], in0=b_bc, in1=mask_sb[:, :, :]
            ).then_inc(dve_sem, 1)

        @blk.gpsimd
        def _(gpsimd):
            gpsimd.memset(ident_sb[:, :], 0.0).then_inc(id_sem, 1)
            gpsimd.memset(mask_sb[:, :, :], 0.0).then_inc(id_sem, 1)
            gpsimd.wait_ge(id_sem, 2)
            gpsimd.affine_select(
                out=ident_sb[:, :],
                in_=ident_sb[:, :],
                compare_op=mybir.AluOpType.not_equal,
                fill=1.0,
                base=0,
                pattern=[[-1, 128]],
                channel_multiplier=1,
            ).then_inc(id_sem, 1)
            gpsimd.affine_select(
                out=mask_sb[:, :, :],
                in_=mask_sb[:, :, :],
                compare_op=mybir.AluOpType.not_equal,
                fill=1.0,
                base=0,
                pattern=[[-1, BS], [0, W]],
                channel_multiplier=1,
            ).then_inc(id_sem, 1)
            # tiny bias loads on the idle GpSimd SWDGE ring
            gpsimd.dma_start(
                out=bab_sb[0:1, 0:H], in_=ba_t[None, :]
            ).then_inc(w_sems[3], 16)
            gpsimd.dma_start(
                out=bab_sb[0:1, H : H + W], in_=bb_t[None, :]
            ).then_inc(w_sems[4], 16)
            if debug_taps:
                gpsimd.wait_ge(act_sem, 5)
                taps = [
                    (dbg["dbg_ysum"], ysum_sb[:, :]),
                    (dbg["dbg_ysum_bf"], ysum_bf[:, :]),
                    (dbg["dbg_hT"],
                     hT_sb[:, :, :, :].rearrange("p x g b -> p (x g b)")),
                    (dbg["dbg_yp"], yp_sb[:, :]),
                    (dbg["dbg_ypT"],
                     ypT_sb[:, :, :].rearrange("p k b -> p (k b)")),
                    (dbg["dbg_ab"], ab_sb[:, :]),
                    (dbg["dbg_bdiag"],
                     bdiag_sb[:, :, :].rearrange("b bb w -> b (bb w)")),
                ]
                dbg_sem = nc.alloc_semaphore("dbg_sem")
                for i, (dst, src_ap) in enumerate(taps):
                    gpsimd.dma_start(out=dst[:, :], in_=src_ap).then_inc(
                        dbg_sem, 16
                    )
                    gpsimd.wait_ge(dbg_sem, 16 * (i + 1))

        @blk.tensor
        def _(pe):
            pe.wait_ge(w_sems[0], 16)
            # mm1T chunks 0..6 during the stream: hT[g] += W1[cc,g]^T @ ysumT
            for cc in range(7):
                pe.wait_ge(cast_sem, cc + 1)
                for g in range(NQH):
                    bank = ps_hte if g % 2 == 0 else ps_hto
                    nc.tensor.matmul(
                        bank[:, g // 2, :],
                        w1_sb[:, cc, g * 128 : (g + 1) * 128],
                        ysum_bf[:, cc * BS : (cc + 1) * BS],
                        start=(cc == 0),
                        stop=False,
                    ).then_inc(pe_sem, 1)
            # PE clock warm (HAM): long burst through the stream tail so
            # the ramp completes, short top-up paced by the last x tile
            pe.wait_ge(id_sem, 4)
            pe.wait_ge(ones_sem, 1)
            red_wait(pe, 6)
            for _i in range(40):
                nc.tensor.matmul(
                    ps_ab[:, :], ident_sb[:, 0:BS], ident_sb[:, 0 : H + W],
                    start=True, stop=True,
                )
            pe.wait_ge(xdma_sems[NT - 1], 16)
            for _i in range(4):
                nc.tensor.matmul(
                    ps_ab[:, :], ident_sb[:, 0:BS], ident_sb[:, 0 : H + W],
                    start=True, stop=True,
                )
            # mm1T chunk 7 (ticks 29..32)
            pe.wait_ge(cast_sem, 8)
            for g in range(NQH):
                bank = ps_hte if g % 2 == 0 else ps_hto
                nc.tensor.matmul(
                    bank[:, g // 2, :],
                    w1_sb[:, 7, g * 128 : (g + 1) * 128],
                    ysum_bf[:, 7 * BS : 8 * BS],
                    start=False,
                    stop=True,
                ).then_inc(pe_sem, 1)
            # mm2: yp[b, :] = hT^T @ W2   (ticks 33..40)
            pe.wait_ge(w_sems[1], 16)
            for half in range(2):
                for q in range(NQH):
                    if half == 0 and q < 2:
                        pe.wait_ge(act_sem, q + 1)
                    nc.tensor.matmul(
                        (ps_yp1 if half == 0 else ps_yp2)[:, :],
                        hT_sb[:, q % 2, q // 2, :],
                        w2_sb[:, q, half * 512 : (half + 1) * 512],
                        start=(q == 0),
                        stop=(q == NQH - 1),
                    ).then_inc(pe_sem, 1)
            # yp transposes + mm3 interleaved (ticks 41..57)
            pe.wait_ge(w_sems[2], 16)
            tr_banks = [tp_a, tp_a, tp_b, tp_b, tp_a, tp_a, tp_b, tp_b]

            def tr(k):
                if k == 0:
                    pe.wait_ge(act_sem, 3)
                if k == 4:
                    pe.wait_ge(act_sem, 4)
                    pe.wait_ge(dve_sem, 1)   # tp_a drained
                if k == 6:
                    pe.wait_ge(dve_sem, 2)   # tp_b drained
                nc.tensor.transpose(
                    tr_banks[k][:, k % 2, :],
                    yp_sb[:, k * 128 : (k + 1) * 128],
                    ident_sb[:BS, :BS],
                ).then_inc(pe_sem, 1)

            def m3(k, copy_idx):
                pe.wait_ge(dve_sem, copy_idx)
                nc.tensor.matmul(
                    ps_ab[:, :],
                    ypT_sb[:, k, :],
                    wab_sb[:, k, :],
                    start=(k == 0),
                    stop=False,
                ).then_inc(pe_sem, 1)

            tr(0); tr(1); tr(2); tr(3)
            m3(0, 1); m3(1, 1)
            tr(4); tr(5)
            m3(2, 2); m3(3, 2)
            tr(6); tr(7)
            m3(4, 3); m3(5, 3)
            m3(6, 4); m3(7, 4)
            pe.wait_ge(w_sems[3], 16)
            pe.wait_ge(w_sems[4], 16)
            nc.tensor.matmul(
                ps_ab[:, :], ones_sb[:, :], bab_sb[:, :],
                start=False, stop=True,
            ).then_inc(pe_sem, 1)
            # outer product vs block-diag Bv (tick 58)
            pe.wait_ge(dve_sem, 6)
            nc.tensor.matmul(
                ps_at[:, :, :].rearrange("h b w -> h (b w)"),
                ab_sb[:, 0:H],
                bdiag_sb[:, :, :].rearrange("b bb w -> b (bb w)"),
                start=True, stop=True,
            ).then_inc(pe_sem, 1)

        @blk.scalar
        def _(act):
            # weights on the scalar HWDGE ring (separate from the x ring)
            act.dma_start(
                out=w1_sb[:, :, :],
                in_=w1_t[:, :].rearrange("p (n h) -> p n h", n=NCC),
            ).then_inc(w_sems[0], 16)
            # dummy gelu loads the ACT table early, off the critical path
            zero = nc.const_aps.aps[(F32, 0.0)]
            nc.scalar.activation(scr_sb[0:1, :], zero[0:1, :], gelu_fn)
            # ACT-side reduces (activation accum_out = free-axis row sum)
            # interleaved with the per-chunk ysum casts f32 -> bf16
            for n, (cc, b0, b1, m0, m1) in enumerate(TILES):
                nb = b1 - b0
                if n == 6:
                    # late weight loads: land well before the epilogue
                    act.dma_start(
                        out=w2_sb[:, :, :],
                        in_=w2_t[:, :].rearrange("p (n h) -> p n h", n=NQH),
                    ).then_inc(w_sems[1], 16)
                    act.dma_start(
                        out=wab_sb[:, :, :],
                        in_=wab_t[:, :].rearrange("p (n h) -> p n h", n=NCC),
                    ).then_inc(w_sems[2], 16)
                if m1 - m0 < HW:
                    if m0 != 0:
                        continue          # T10 is DVE's
                    act.wait_ge(xdma_sems[n], 16)
                    nc.scalar.activation(   # T9 first half -> col 31
                        red_scr[:, 0 : m1 - m0],
                        x_sb[:, n % NSLOT, 0, 0 : m1 - m0],
                        copy_fn,
                        accum_out=ysum_sb[:, 31:32],
                    ).then_inc(red_o, 1)
                    continue
                ne = (nb + 1) // 2
                act.wait_ge(xdma_sems[n], 16)
                for j in range(ne, nb):   # ACT: blocks ne..nb-1, one each
                    nc.scalar.activation(
                        red_scr[:, :],
                        x_sb[:, n % NSLOT, j, :],
                        copy_fn,
                        accum_out=ysum_sb[:, cc * BS + b0 + j : cc * BS
                                          + b0 + j + 1],
                    ).then_inc(red_o, 1)
                if cc < 7 and b1 == 4:
                    # cast cc: self-wait drains own reduce writes (RAW)
                    act.wait_ge(red_o, RO[n])
                    act.wait_ge(red_e, RE[n])
                    nc.scalar.copy(
                        out=ysum_bf[:, cc * BS : (cc + 1) * BS],
                        in_=ysum_sb[:, cc * BS : (cc + 1) * BS],
                    ).then_inc(cast_sem, 1)
            act.wait_ge(red_o, RO[NT - 1])
            act.wait_ge(add_sem, 1)
            nc.scalar.copy(
                out=ysum_bf[:, 28:32], in_=ysum_sb[:, 28:32]
            ).then_inc(cast_sem, 1)
            # gelu on h^T straight out of PSUM (scale folds in the 1/HW)
            act.wait_ge(pe_sem, 31)
            nc.scalar.activation(
                hT_sb[:, 0, :, :], ps_hte[:, :, :], gelu_fn, scale=1.0 / HW
            ).then_inc(act_sem, 1)
            act.wait_ge(pe_sem, 32)
            nc.scalar.activation(
                hT_sb[:, 1, :, :], ps_hto[:, :, :], gelu_fn, scale=1.0 / HW
            ).then_inc(act_sem, 1)
            act.wait_ge(pe_sem, 36)
            nc.scalar.activation(
                yp_sb[:, 0 : C // 2], ps_yp1[:, :], gelu_fn
            ).then_inc(act_sem, 1)
            act.wait_ge(pe_sem, 40)
            nc.scalar.activation(
                yp_sb[:, C // 2 : C], ps_yp2[:, :], gelu_fn
            ).then_inc(act_sem, 1)
            # dummy sigmoid swaps the ACT table while PE transposes run
            nc.scalar.activation(scr_sb[0:1, :], zero[0:1, :], sig_fn)
            act.wait_ge(pe_sem, 58)
            nc.scalar.activation(
                attn_sb[:, :, :], ps_at[:, :, :], sig_fn
            ).then_inc(act_sem, 1)

    return nc


_NC_CACHE: list = []


def run_on_hw(x, W1, W2, WA, bA, WB, bB, **spmd_kwargs):
    """Run the SPMD kernel; returns (full_output, BassKernelResults)."""
    import ml_dtypes

    bf = ml_dtypes.bfloat16
    x = np.ascontiguousarray(np.asarray(x, dtype=np.float32))
    # pack weights into SBUF layout: [p, n, ...] where row n*128+p -> (p, n)
    w1p = np.asarray(W1).reshape(NCC, 128, HID).transpose(1, 0, 2)
    w2p = np.asarray(W2).reshape(NQH, 128, C).transpose(1, 0, 2)
    wab = np.concatenate([np.asarray(WA), np.asarray(WB)], axis=1)  # (C, H+W)
    wabp = wab.reshape(NCC, 128, H + W).transpose(1, 0, 2)
    weights = {
        "W1p": np.ascontiguousarray(w1p.reshape(128, NCC * HID).astype(bf)),
        "W2p": np.ascontiguousarray(w2p.reshape(128, NQH * C).astype(bf)),
        "WABp": np.ascontiguousarray(
            wabp.reshape(128, NCC * (H + W)).astype(bf)
        ),
        "bAbf": np.ascontiguousarray(np.asarray(bA).astype(bf)),
        "bBbf": np.ascontiguousarray(np.asarray(bB).astype(bf)),
    }

    if not _NC_CACHE:
        _NC_CACHE.append(build_bass())
    nc = _NC_CACHE[0]

    in_maps = []
    for i in range(NCORES):
        shard = x[i * BS : (i + 1) * BS].reshape(ROWS, HW)
        in_maps.append({"x": shard, **weights})

    res = run_bass_kernel_spmd(
        nc, in_maps, core_ids=list(range(NCORES)), **spmd_kwargs
    )
    # per-core out is [H, BS*W]; swap to (BS, H, W) then concat cores
    attn = np.concatenate(
        [r["out"].reshape(H, BS, W).transpose(1, 0, 2) for r in res.results],
        axis=0,
    )  # (B, H, W)
    return np.broadcast_to(attn.reshape(B, 1, H, W), (B, C, H, W)), res


def kernel(x, W1, W2, WA, bA, WB, bB):
    out, _ = run_on_hw(x, W1, W2, WA, bA, WB, bB)
    return out


# revision 39
# speedup vs baseline: 1.1487x; 1.1487x over previous
"""Trainium2 Bass kernel for nn_AdaptiveBlock (dense_mlp).

Reference computation:
    y    = mean(x, axis=(2, 3))                   # (B, C) global avg pool
    h    = gelu(y @ W1)                           # (B, HID), exact erf gelu
    yp   = gelu(h @ W2)                           # (B, C)
    A    = yp @ WA + bA                           # (B, H)
    Bv   = yp @ WB + bB                           # (B, W)
    attn = sigmoid(A[:,None,:,None] * Bv[:,None,None,:])   # (B, 1, H, W)
    out  = broadcast(attn, (B, C, H, W))

Sharding: data-parallel over batch across 8 NeuronCores (4 batches/core),
weights replicated, no collectives.  Each core streams its 51.4 MB x-shard
through SBUF, row-reduces it, runs the tiny MLP on-chip, and writes only
its (56, 4*56) attention map.  The channel broadcast (and the h<->b axis
swap) is done on the host; it carries no information.

Key design points (measured on HW, see git history of this file):
  * x streams CHUNK-major: each 4-block tile completes one contraction
    chunk of mm1, so mm1 runs DURING the stream.  mm1 is computed
    TRANSPOSED (lhsT = W1 chunk [128c x 128hid], rhs = ysumT [128c x 4b]),
    which also yields h^T directly -- no PE transposes of h before mm2.
  * The x DMA issue is gated to <=3 outstanding tiles.  A deep HWDGE ring
    makes SDMA engine 15 run ~20% slow (starving every tile's completion
    semaphore); a shallow ring keeps the engines balanced.
  * Each tile's reduce is split DVE (blocks 0-1) / ACT accum_out (blocks
    2-3), halving per-tile reduce latency so the DMA gate always clears
    early and the stream never stalls.
  * Weights ride the SCALAR HWDGE ring (second hardware ring, full rate,
    no SWDGE interleave penalty); host pre-packs them into SBUF layout so
    each weight DMA is one big contiguous descriptor per partition.
  * Tail tiles shrink to 2/1/half blocks so the last reduce is ~1.7us.
  * Same-engine back-to-back RAW through SBUF is NOT interlocked (relaxed
    ordering): every such hazard carries an explicit self-wait.
  * Single sigmoid, single contiguous 50KB output DMA ([H, BS*W] layout).
"""

import numpy as np

import concourse.bass as bass
from concourse import mybir
from concourse.bass_utils import run_bass_kernel_spmd

B, C, HID, H, W = 32, 1024, 512, 56, 56
NCORES = 8
BS = B // NCORES          # 4 batches per core
ROWS = BS * C             # 4096 (b, c) rows per core
HW = H * W                # 3136
NCC = C // 128            # 8 contraction chunks for mm1
NQH = HID // 128          # 4 contraction chunks for mm2
NSLOT = 3                 # x slots (4 blocks each)
F32 = mybir.dt.float32
BF16 = mybir.dt.bfloat16

# x tiles, chunk-major: (cc, b_lo, b_hi, m_lo, m_hi).
# T0..T6 = chunks 0-6 (4 blocks); chunk 7 tapers: 2 + 1 + half + half.
TILES = [(cc, 0, 4, 0, HW) for cc in range(7)] + [
    (7, 0, 2, 0, HW),
    (7, 2, 3, 0, HW),
    (7, 3, 4, 0, HW // 2),
    (7, 3, 4, HW // 2, HW),
]
NT = len(TILES)  # 11

# per-tile reduce split: DVE takes the first ceil(nb/2) blocks, ACT the
# rest -- except the half-block tiles T9 (ACT) / T10 (DVE).
# cumulative (red_e, red_o) counts after tile m's reduce is done:
RE, RO = [], []
e = o = 0
for m, (cc, b0, b1, m0, m1) in enumerate(TILES):
    nb = b1 - b0
    if m1 - m0 < HW:          # half tiles: T9 on ACT, T10 on DVE
        if m0 == 0:
            o += 1
        else:
            e += 1
    else:
        e += 1                # DVE: one reduce over its block span
        o += nb - (nb + 1) // 2   # ACT: one accum per block
    RE.append(e)
    RO.append(o)


def build_bass(gelu_fn=None, debug_taps=False) -> bass.Bass:
    if gelu_fn is None:
        gelu_fn = mybir.ActivationFunctionType.Gelu
    sig_fn = mybir.ActivationFunctionType.Sigmoid
    copy_fn = mybir.ActivationFunctionType.Copy
    nc = bass.Bass()

    x_t = nc.dram_tensor("x", [ROWS, HW], F32, kind="ExternalInput")
    w1_t = nc.dram_tensor("W1p", [128, NCC * HID], BF16, kind="ExternalInput")
    w2_t = nc.dram_tensor("W2p", [128, NQH * C], BF16, kind="ExternalInput")
    wab_t = nc.dram_tensor(
        "WABp", [128, NCC * (H + W)], BF16, kind="ExternalInput"
    )
    ba_t = nc.dram_tensor("bAbf", [H], BF16, kind="ExternalInput")
    bb_t = nc.dram_tensor("bBbf", [W], BF16, kind="ExternalInput")
    # [h, (b w)] layout; host swaps to (b, h, w)
    out_t = nc.dram_tensor("out", [H, BS * W], F32, kind="ExternalOutput")
    dbg = {}
    if debug_taps:
        for name, shape in [
            ("dbg_ysum", [128, 33]), ("dbg_ysum_bf", [128, 32]),
            ("dbg_hT", [128, 2 * 2 * BS]), ("dbg_yp", [BS, C]),
            ("dbg_ypT", [128, NCC * BS]), ("dbg_ab", [BS, H + W]),
            ("dbg_bdiag", [BS, BS * W]),
        ]:
            dbg[name] = nc.dram_tensor(name, shape, F32, kind="ExternalOutput")

    # x row r = b*C + cc*128 + p; chunk-major view [cc, p, b, m]
    x_r = x_t[:, :].rearrange("(b q p) m -> q p b m", b=BS, q=NCC, p=128)

    # ---- SBUF ----
    x_sb = nc.alloc_sbuf_tensor("x_sb", [128, NSLOT, BS, HW], F32)
    ysum_sb = nc.alloc_sbuf_tensor("ysum_sb", [128, 33], F32)  # col32 scratch
    ysum_bf = nc.alloc_sbuf_tensor("ysum_bf", [128, 32], BF16)
    w1_sb = nc.alloc_sbuf_tensor("w1_sb", [128, NCC, HID], BF16)
    w2_sb = nc.alloc_sbuf_tensor("w2_sb", [128, NQH, C], BF16)
    wab_sb = nc.alloc_sbuf_tensor("wab_sb", [128, NCC, H + W], BF16)
    bab_sb = nc.alloc_sbuf_tensor("bab_sb", [1, H + W], BF16)
    ident_sb = nc.alloc_sbuf_tensor("ident_sb", [128, 128], BF16)
    ones_sb = nc.alloc_sbuf_tensor("ones_sb", [1, BS], BF16)
    mask_sb = nc.alloc_sbuf_tensor("mask_sb", [BS, BS, W], BF16)
    # hT layout [p, parity, gg, b]: hid group g = 2*gg + parity, so the
    # even/odd-bank gelus each write one contiguous slice
    hT_sb = nc.alloc_sbuf_tensor("hT_sb", [128, 2, 2, BS], BF16)
    yp_sb = nc.alloc_sbuf_tensor("yp_sb", [BS, C], BF16)
    ypT_sb = nc.alloc_sbuf_tensor("ypT_sb", [128, NCC, BS], BF16)
    ab_sb = nc.alloc_sbuf_tensor("ab_sb", [BS, H + W], BF16)
    bdiag_sb = nc.alloc_sbuf_tensor("bdiag_sb", [BS, BS, W], BF16)
    attn_sb = nc.alloc_sbuf_tensor("attn_sb", [H, BS, W], F32)
    scr_sb = nc.alloc_sbuf_tensor("scr_sb", [1, 1], F32)
    # dump target for the ACT-side reduces (activation must write a full
    # output even when only accum_out is wanted)
    red_scr = nc.alloc_sbuf_tensor("red_scr", [128, HW], BF16)

    # ---- PSUM (8 banks) ----
    # hT accumulators: even groups (0,2) / odd groups (1,3) in separate
    # banks so a gelu read never overlaps a PE write to the same bank.
    ps_hte = nc.alloc_psum_tensor("ps_hte", [128, 2, BS], F32)
    ps_hto = nc.alloc_psum_tensor("ps_hto", [128, 2, BS], F32)
    ps_yp1 = nc.alloc_psum_tensor("ps_yp1", [BS, C // 2], F32)
    ps_yp2 = nc.alloc_psum_tensor("ps_yp2", [BS, C // 2], F32)
    tp_a = nc.alloc_psum_tensor("tp_a", [128, 2, BS], BF16)
    tp_b = nc.alloc_psum_tensor("tp_b", [128, 2, BS], BF16)
    ps_ab = nc.alloc_psum_tensor("ps_ab", [BS, H + W], F32)
    ps_at = nc.alloc_psum_tensor("ps_at", [H, BS, W], F32)

    # ---- semaphores ----
    xdma_sems = [nc.alloc_semaphore(f"xdma_sem{n}") for n in range(NT)]
    w_sems = [nc.alloc_semaphore(f"w_sem{i}") for i in range(5)]
    id_sem = nc.alloc_semaphore("id_sem")
    ones_sem = nc.alloc_semaphore("ones_sem")
    red_e = nc.alloc_semaphore("red_e")        # DVE reduce progress
    red_o = nc.alloc_semaphore("red_o")        # ACT reduce progress
    add_sem = nc.alloc_semaphore("add_sem")    # last half-block folded in
    cast_sem = nc.alloc_semaphore("cast_sem")  # +1 per chunk cast (ACT)
    pe_sem = nc.alloc_semaphore("pe_sem")
    act_sem = nc.alloc_semaphore("act_sem")
    dve_sem = nc.alloc_semaphore("dve_sem")
    out_sem = nc.alloc_semaphore("out_sem")

    def red_wait(eng, m):
        """Wait until tile m's reduce is complete on both engines."""
        eng.wait_ge(red_e, RE[m])
        eng.wait_ge(red_o, RO[m])

    # PE ticks (pe_sem after every real PE op):
    #   1..28  mm1T chunks 0..6 (4 pairs each)
    #   29..32 mm1T chunk 7, groups g=0..3
    #   33..36 mm2 half0 q=0..3     37..40 mm2 half1 q=0..3
    #   41..44 tr0..tr3   45,46 m0,m1   47,48 tr4,tr5   49,50 m2,m3
    #   51,52 tr6,tr7   53,54 m4,m5   55,56 m6,m7   57 bias   58 outer
    # ACT increments (act_sem): gelu_hT_even 1, gelu_hT_odd 2,
    #   gelu_yp1 3, gelu_yp2 4, sigmoid 5
    # DVE increments (dve_sem): ypT copies 1..4, ab copy 5, bdiag 6

    with nc.Block() as blk:

        @blk.sync
        def _(sync):
            for n, (cc, b0, b1, m0, m1) in enumerate(TILES):
                if n >= NSLOT:
                    red_wait(sync, n - NSLOT)
                sync.dma_start(
                    out=x_sb[:, n % NSLOT, 0 : b1 - b0, 0 : m1 - m0],
                    in_=x_r[cc, :, b0:b1, m0:m1],
                ).then_inc(xdma_sems[n], 16)
            sync.wait_ge(act_sem, 5)
            sync.dma_start(
                out=out_t[:, :],
                in_=attn_sb[:, :, :].rearrange("h b w -> h (b w)"),
            ).then_inc(out_sem, 16)
            sync.wait_ge(out_sem, 16)

        @blk.vector
        def _(vec):
            vec.memset(ones_sb[:, :], 1.0).then_inc(ones_sem, 1)
            for n, (cc, b0, b1, m0, m1) in enumerate(TILES):
                nb = b1 - b0
                if m1 - m0 < HW:
                    if m0 == 0:
                        continue          # T9 is ACT's
                    vec.wait_ge(xdma_sems[n], 16)
                    vec.reduce_sum(       # T10 second half -> scratch col
                        out=ysum_sb[:, 32:33],
                        in_=x_sb[:, n % NSLOT, 0:1, 0 : m1 - m0],
                        axis=mybir.AxisListType.X,
                    ).then_inc(red_e, 1)
                    continue
                ne = (nb + 1) // 2        # DVE's share: first ne blocks
                vec.wait_ge(xdma_sems[n], 16)
                vec.reduce_sum(
                    out=ysum_sb[:, cc * BS + b0 : cc * BS + b0 + ne],
                    in_=x_sb[:, n % NSLOT, 0:ne, :],
                    axis=mybir.AxisListType.X,
                ).then_inc(red_e, 1)
            # fold the halves: col31 (ACT, T9) + col32 (own T10).
            # self-wait: same-engine RAW through SBUF is not interlocked
            vec.wait_ge(red_e, RE[NT - 1])
            vec.wait_ge(red_o, RO[NT - 1])
            nc.vector.tensor_add(
                out=ysum_sb[:, 31:32],
                in0=ysum_sb[:, 31:32],
                in1=ysum_sb[:, 32:33],
            ).then_inc(add_sem, 1)
            # ypT copies: tp_a{tr0,tr1}, tp_b{tr2,tr3}, tp_a{tr4,tr5}, ...
            for i, (bank, tick) in enumerate(
                [(tp_a, 42), (tp_b, 44), (tp_a, 48), (tp_b, 52)]
            ):
                vec.wait_ge(pe_sem, tick)
                nc.vector.tensor_copy(
                    out=ypT_sb[:, 2 * i : 2 * i + 2, :], in_=bank[:, :, :]
                ).then_inc(dve_sem, 1)
            vec.wait_ge(pe_sem, 57)
            nc.vector.tensor_copy(
                out=ab_sb[:, :], in_=ps_ab[:, :]
            ).then_inc(dve_sem, 1)
            # self-wait: the bdiag mul reads ab_sb the copy just wrote
            vec.wait_ge(dve_sem, 5)
            vec.wait_ge(id_sem, 4)
            # bdiag[b, bb, w] = Bv[b, w] * (b == bb)
            b_sl = ab_sb[:, H : H + W]
            b_bc = bass.AP(
                tensor=b_sl.tensor, offset=b_sl.offset,
                ap=[b_sl.ap[0], [0, BS], [b_sl.ap[1][0], W]],
            )
            nc.vector.tensor_mul(
                out=bdiag_sb[:, :, :], in0=b_bc, in1=mask_sb[:, :, :]
            ).then_inc(dve_sem, 1)

        @blk.gpsimd
        def _(gpsimd):
            gpsimd.memset(ident_sb[:, :], 0.0).then_inc(id_sem, 1)
            gpsimd.memset(mask_sb[:, :, :], 0.0).then_inc(id_sem, 1)
            gpsimd.wait_ge(id_sem, 2)
            gpsimd.affine_select(
                out=ident_sb[:, :],
                in_=ident_sb[:, :],
                compare_op=mybir.AluOpType.not_equal,
                fill=1.0,
                base=0,
                pattern=[[-1, 128]],
                channel_multiplier=1,
            ).then_inc(id_sem, 1)
            gpsimd.affine_select(
                out=mask_sb[:, :, :],
                in_=mask_sb[:, :, :],
                compare_op=mybir.AluOpType.not_equal,
                fill=1.0,
                base=0,
                pattern=[[-1, BS], [0, W]],
                channel_multiplier=1,
            ).then_inc(id_sem, 1)
            # all weight loads on the GpSimd SWDGE ring: HWDGE-ring weight
            # DMAs correlate with a persistent ~20% throttle of SDMA
            # engine 15, which starves every x-tile completion semaphore
            gpsimd.dma_start(
                out=w1_sb[:, :, :],
                in_=w1_t[:, :].rearrange("p (n h) -> p n h", n=NCC),
            ).then_inc(w_sems[0], 16)
            gpsimd.dma_start(
                out=bab_sb[0:1, 0:H], in_=ba_t[None, :]
            ).then_inc(w_sems[3], 16)
            gpsimd.dma_start(
                out=bab_sb[0:1, H : H + W], in_=bb_t[None, :]
            ).then_inc(w_sems[4], 16)
            # W2/WAB gated to the stream tail (SWDGE steals bandwidth
            # 1:1-ish while x streams, so load them as the stream winds
            # down -- they are only needed at the epilogue)
            gpsimd.wait_ge(red_e, RE[5])
            gpsimd.dma_start(
                out=w2_sb[:, :, :],
                in_=w2_t[:, :].rearrange("p (n h) -> p n h", n=NQH),
            ).then_inc(w_sems[1], 16)
            gpsimd.dma_start(
                out=wab_sb[:, :, :],
                in_=wab_t[:, :].rearrange("p (n h) -> p n h", n=NCC),
            ).then_inc(w_sems[2], 16)
            if debug_taps:
                gpsimd.wait_ge(act_sem, 5)
                taps = [
                    (dbg["dbg_ysum"], ysum_sb[:, :]),
                    (dbg["dbg_ysum_bf"], ysum_bf[:, :]),
                    (dbg["dbg_hT"],
                     hT_sb[:, :, :, :].rearrange("p x g b -> p (x g b)")),
                    (dbg["dbg_yp"], yp_sb[:, :]),
                    (dbg["dbg_ypT"],
                     ypT_sb[:, :, :].rearrange("p k b -> p (k b)")),
                    (dbg["dbg_ab"], ab_sb[:, :]),
                    (dbg["dbg_bdiag"],
                     bdiag_sb[:, :, :].rearrange("b bb w -> b (bb w)")),
                ]
                dbg_sem = nc.alloc_semaphore("dbg_sem")
                for i, (dst, src_ap) in enumerate(taps):
                    gpsimd.dma_start(out=dst[:, :], in_=src_ap).then_inc(
                        dbg_sem, 16
                    )
                    gpsimd.wait_ge(dbg_sem, 16 * (i + 1))

        @blk.tensor
        def _(pe):
            pe.wait_ge(w_sems[0], 16)
            # mm1T chunks 0..6 during the stream: hT[g] += W1[cc,g]^T @ ysumT
            for cc in range(7):
                pe.wait_ge(cast_sem, cc + 1)
                for g in range(NQH):
                    bank = ps_hte if g % 2 == 0 else ps_hto
                    nc.tensor.matmul(
                        bank[:, g // 2, :],
                        w1_sb[:, cc, g * 128 : (g + 1) * 128],
                        ysum_bf[:, cc * BS : (cc + 1) * BS],
                        start=(cc == 0),
                        stop=False,
                    ).then_inc(pe_sem, 1)
            # PE clock warm (HAM): long burst through the stream tail so
            # the ramp completes, short top-up paced by the last x tile
            pe.wait_ge(id_sem, 4)
            pe.wait_ge(ones_sem, 1)
            red_wait(pe, 6)
            for _i in range(40):
                nc.tensor.matmul(
                    ps_ab[:, :], ident_sb[:, 0:BS], ident_sb[:, 0 : H + W],
                    start=True, stop=True,
                )
            pe.wait_ge(xdma_sems[NT - 1], 16)
            for _i in range(4):
                nc.tensor.matmul(
                    ps_ab[:, :], ident_sb[:, 0:BS], ident_sb[:, 0 : H + W],
                    start=True, stop=True,
                )
            # mm1T chunk 7 (ticks 29..32)
            pe.wait_ge(cast_sem, 8)
            for g in range(NQH):
                bank = ps_hte if g % 2 == 0 else ps_hto
                nc.tensor.matmul(
                    bank[:, g // 2, :],
                    w1_sb[:, 7, g * 128 : (g + 1) * 128],
                    ysum_bf[:, 7 * BS : 8 * BS],
                    start=False,
                    stop=True,
                ).then_inc(pe_sem, 1)
            # mm2: yp[b, :] = hT^T @ W2   (ticks 33..40)
            pe.wait_ge(w_sems[1], 16)
            for half in range(2):
                for q in range(NQH):
                    if half == 0 and q < 2:
                        pe.wait_ge(act_sem, q + 1)
                    nc.tensor.matmul(
                        (ps_yp1 if half == 0 else ps_yp2)[:, :],
                        hT_sb[:, q % 2, q // 2, :],
                        w2_sb[:, q, half * 512 : (half + 1) * 512],
                        start=(q == 0),
                        stop=(q == NQH - 1),
                    ).then_inc(pe_sem, 1)
            # yp transposes + mm3 interleaved (ticks 41..57)
            pe.wait_ge(w_sems[2], 16)
            tr_banks = [tp_a, tp_a, tp_b, tp_b, tp_a, tp_a, tp_b, tp_b]

            def tr(k):
                if k == 0:
                    pe.wait_ge(act_sem, 3)
                if k == 4:
                    pe.wait_ge(act_sem, 4)
                    pe.wait_ge(dve_sem, 1)   # tp_a drained
                if k == 6:
                    pe.wait_ge(dve_sem, 2)   # tp_b drained
                nc.tensor.transpose(
                    tr_banks[k][:, k % 2, :],
                    yp_sb[:, k * 128 : (k + 1) * 128],
                    ident_sb[:BS, :BS],
                ).then_inc(pe_sem, 1)

            def m3(k, copy_idx):
                pe.wait_ge(dve_sem, copy_idx)
                nc.tensor.matmul(
                    ps_ab[:, :],
                    ypT_sb[:, k, :],
                    wab_sb[:, k, :],
                    start=(k == 0),
                    stop=False,
                ).then_inc(pe_sem, 1)

            tr(0); tr(1); tr(2); tr(3)
            m3(0, 1); m3(1, 1)
            tr(4); tr(5)
            m3(2, 2); m3(3, 2)
            tr(6); tr(7)
            m3(4, 3); m3(5, 3)
            m3(6, 4); m3(7, 4)
            pe.wait_ge(w_sems[3], 16)
            pe.wait_ge(w_sems[4], 16)
            nc.tensor.matmul(
                ps_ab[:, :], ones_sb[:, :], bab_sb[:, :],
                start=False, stop=True,
            ).then_inc(pe_sem, 1)
            # outer product vs block-diag Bv (tick 58)
            pe.wait_ge(dve_sem, 6)
            nc.tensor.matmul(
                ps_at[:, :, :].rearrange("h b w -> h (b w)"),
                ab_sb[:, 0:H],
                bdiag_sb[:, :, :].rearrange("b bb w -> b (bb w)"),
                start=True, stop=True,
            ).then_inc(pe_sem, 1)

        @blk.scalar
        def _(act):
            # dummy gelu loads the ACT table early, off the critical path
            zero = nc.const_aps.aps[(F32, 0.0)]
            nc.scalar.activation(scr_sb[0:1, :], zero[0:1, :], gelu_fn)
            # ACT-side reduces (activation accum_out = free-axis row sum)
            # interleaved with the per-chunk ysum casts f32 -> bf16
            for n, (cc, b0, b1, m0, m1) in enumerate(TILES):
                nb = b1 - b0
                if m1 - m0 < HW:
                    if m0 != 0:
                        continue          # T10 is DVE's
                    act.wait_ge(xdma_sems[n], 16)
                    nc.scalar.activation(   # T9 first half -> col 31
                        red_scr[:, 0 : m1 - m0],
                        x_sb[:, n % NSLOT, 0, 0 : m1 - m0],
                        copy_fn,
                        accum_out=ysum_sb[:, 31:32],
                    ).then_inc(red_o, 1)
                    continue
                ne = (nb + 1) // 2
                act.wait_ge(xdma_sems[n], 16)
                for j in range(ne, nb):   # ACT: blocks ne..nb-1, one each
                    nc.scalar.activation(
                        red_scr[:, :],
                        x_sb[:, n % NSLOT, j, :],
                        copy_fn,
                        accum_out=ysum_sb[:, cc * BS + b0 + j : cc * BS
                                          + b0 + j + 1],
                    ).then_inc(red_o, 1)
                if cc < 7 and b1 == 4:
                    # cast cc: self-wait drains own reduce writes (RAW)
                    act.wait_ge(red_o, RO[n])
                    act.wait_ge(red_e, RE[n])
                    nc.scalar.copy(
                        out=ysum_bf[:, cc * BS : (cc + 1) * BS],
                        in_=ysum_sb[:, cc * BS : (cc + 1) * BS],
                    ).then_inc(cast_sem, 1)
            act.wait_ge(red_o, RO[NT - 1])
            act.wait_ge(add_sem, 1)
            nc.scalar.copy(
                out=ysum_bf[:, 28:32], in_=ysum_sb[:, 28:32]
            ).then_inc(cast_sem, 1)
            # gelu on h^T straight out of PSUM (scale folds in the 1/HW)
            act.wait_ge(pe_sem, 31)
            nc.scalar.activation(
                hT_sb[:, 0, :, :], ps_hte[:, :, :], gelu_fn, scale=1.0 / HW
            ).then_inc(act_sem, 1)
            act.wait_ge(pe_sem, 32)
            nc.scalar.activation(
                hT_sb[:, 1, :, :], ps_hto[:, :, :], gelu_fn, scale=1.0 / HW
            ).then_inc(act_sem, 1)
            act.wait_ge(pe_sem, 36)
            nc.scalar.activation(
                yp_sb[:, 0 : C // 2], ps_yp1[:, :], gelu_fn
            ).then_inc(act_sem, 1)
            act.wait_ge(pe_sem, 40)
            nc.scalar.activation(
                yp_sb[:, C // 2 : C], ps_yp2[:, :], gelu_fn
            ).then_inc(act_sem, 1)
            # dummy sigmoid swaps the ACT table while PE transposes run
            nc.scalar.activation(scr_sb[0:1, :], zero[0:1, :], sig_fn)
            act.wait_ge(pe_sem, 58)
            nc.scalar.activation(
                attn_sb[:, :, :], ps_at[:, :, :], sig_fn
            ).then_inc(act_sem, 1)

    return nc


_NC_CACHE: list = []


def run_on_hw(x, W1, W2, WA, bA, WB, bB, **spmd_kwargs):
    """Run the SPMD kernel; returns (full_output, BassKernelResults)."""
    import ml_dtypes

    bf = ml_dtypes.bfloat16
    x = np.ascontiguousarray(np.asarray(x, dtype=np.float32))
    # pack weights into SBUF layout: [p, n, ...] where row n*128+p -> (p, n)
    w1p = np.asarray(W1).reshape(NCC, 128, HID).transpose(1, 0, 2)
    w2p = np.asarray(W2).reshape(NQH, 128, C).transpose(1, 0, 2)
    wab = np.concatenate([np.asarray(WA), np.asarray(WB)], axis=1)  # (C, H+W)
    wabp = wab.reshape(NCC, 128, H + W).transpose(1, 0, 2)
    weights = {
        "W1p": np.ascontiguousarray(w1p.reshape(128, NCC * HID).astype(bf)),
        "W2p": np.ascontiguousarray(w2p.reshape(128, NQH * C).astype(bf)),
        "WABp": np.ascontiguousarray(
            wabp.reshape(128, NCC * (H + W)).astype(bf)
        ),
        "bAbf": np.ascontiguousarray(np.asarray(bA).astype(bf)),
        "bBbf": np.ascontiguousarray(np.asarray(bB).astype(bf)),
    }

    if not _NC_CACHE:
        _NC_CACHE.append(build_bass())
    nc = _NC_CACHE[0]

    in_maps = []
    for i in range(NCORES):
        shard = x[i * BS : (i + 1) * BS].reshape(ROWS, HW)
        in_maps.append({"x": shard, **weights})

    res = run_bass_kernel_spmd(
        nc, in_maps, core_ids=list(range(NCORES)), **spmd_kwargs
    )
    # per-core out is [H, BS*W]; swap to (BS, H, W) then concat cores
    attn = np.concatenate(
        [r["out"].reshape(H, BS, W).transpose(1, 0, 2) for r in res.results],
        axis=0,
    )  # (B, H, W)
    return np.broadcast_to(attn.reshape(B, 1, H, W), (B, C, H, W)), res


def kernel(x, W1, W2, WA, bA, WB, bB):
    out, _ = run_on_hw(x, W1, W2, WA, bA, WB, bB)
    return out


# revision 46
# speedup vs baseline: 1.1662x; 1.0152x over previous
"""Trainium2 Bass kernel for nn_AdaptiveBlock (dense_mlp).

Reference computation:
    y    = mean(x, axis=(2, 3))                   # (B, C) global avg pool
    h    = gelu(y @ W1)                           # (B, HID), exact erf gelu
    yp   = gelu(h @ W2)                           # (B, C)
    A    = yp @ WA + bA                           # (B, H)
    Bv   = yp @ WB + bB                           # (B, W)
    attn = sigmoid(A[:,None,:,None] * Bv[:,None,None,:])   # (B, 1, H, W)
    out  = broadcast(attn, (B, C, H, W))

Sharding: data-parallel over batch across 8 NeuronCores (4 batches/core),
weights replicated, no collectives.  Each core streams its 51.4 MB x-shard
through SBUF, row-reduces it, runs the tiny MLP on-chip, and writes only
its (56, 4*56) attention map.  The channel broadcast (and the h<->b axis
swap) is done on the host; it carries no information.

Key design points (measured on HW, see git history of this file):
  * x streams CHUNK-major: each 4-block tile completes one contraction
    chunk of mm1, so mm1 runs DURING the stream.  mm1 is computed
    TRANSPOSED (lhsT = W1 chunk [128c x 128hid], rhs = ysumT [128c x 4b]),
    which also yields h^T directly -- no PE transposes of h before mm2.
  * The x DMA issue is gated to <=3 outstanding tiles.  A deep HWDGE ring
    makes SDMA engine 15 run ~20% slow (starving every tile's completion
    semaphore); a shallow ring keeps the engines balanced.
  * Each tile's reduce is split DVE (blocks 0-1) / ACT accum_out (blocks
    2-3), halving per-tile reduce latency so the DMA gate always clears
    early and the stream never stalls.
  * Weights ride the SCALAR HWDGE ring (second hardware ring, full rate,
    no SWDGE interleave penalty); host pre-packs them into SBUF layout so
    each weight DMA is one big contiguous descriptor per partition.
  * Tail tiles shrink to 2/1/half blocks so the last reduce is ~1.7us.
  * Same-engine back-to-back RAW through SBUF is NOT interlocked (relaxed
    ordering): every such hazard carries an explicit self-wait.
  * Single sigmoid, single contiguous 50KB output DMA ([H, BS*W] layout).
"""

import numpy as np

import concourse.bass as bass
from concourse import mybir
from concourse.bass_utils import run_bass_kernel_spmd

B, C, HID, H, W = 32, 1024, 512, 56, 56
NCORES = 8
BS = B // NCORES          # 4 batches per core
ROWS = BS * C             # 4096 (b, c) rows per core
HW = H * W                # 3136
NCC = C // 128            # 8 contraction chunks for mm1
NQH = HID // 128          # 4 contraction chunks for mm2
NSLOT = 3                 # x slots (4 blocks each)
F32 = mybir.dt.float32
BF16 = mybir.dt.bfloat16

# x tiles, chunk-major: (cc, b_lo, b_hi, m_lo, m_hi, slot, sblk, gate).
# T0..T6 = chunks 0-6 (4 blocks); chunk 7 tapers: 2 + 1 + half + half.
# The small chunk-7 tiles pack into free blocks of slots whose previous
# tile is long reduced (gate = index of that previous tile), so the tail
# DMAs issue back-to-back and never wait on the reduce chain.
TILES = [(cc, 0, 4, 0, HW, cc % 3, 0, cc - 3) for cc in range(7)] + [
    (7, 0, 2, 0, HW, 1, 0, 4),        # T7: slot1 blocks 0-1 (after T4)
    (7, 2, 3, 0, HW, 1, 2, 4),        # T8: slot1 block 2
    (7, 3, 4, 0, HW // 2, 2, 0, 5),   # T9: slot2 block 0 (after T5)
    (7, 3, 4, HW // 2, HW, 2, 1, 5),  # T10: slot2 block 1
]
NT = len(TILES)  # 11

# per-tile reduce split: DVE takes the first ceil(nb/2) blocks, ACT the
# rest -- except the half-block tiles T9 (ACT) / T10 (DVE).
# cumulative (red_e, red_o) counts after tile m's reduce is done:
RE, RO = [], []
e = o = 0
for m, (cc, b0, b1, m0, m1, _sl, _sb, _g) in enumerate(TILES):
    nb = b1 - b0
    if m1 - m0 < HW:          # half tiles: T9 on ACT, T10 on DVE
        if m0 == 0:
            o += 1
        else:
            e += 1
    else:
        e += 1                # DVE: one reduce over its block span
        o += nb - (nb + 1) // 2   # ACT: one accum per block
    RE.append(e)
    RO.append(o)


def build_bass(gelu_fn=None, debug_taps=False) -> bass.Bass:
    if gelu_fn is None:
        gelu_fn = mybir.ActivationFunctionType.Gelu
    sig_fn = mybir.ActivationFunctionType.Sigmoid
    copy_fn = mybir.ActivationFunctionType.Copy
    nc = bass.Bass()

    x_t = nc.dram_tensor("x", [ROWS, HW], F32, kind="ExternalInput")
    w1_t = nc.dram_tensor("W1p", [128, NCC * HID], BF16, kind="ExternalInput")
    w2_t = nc.dram_tensor("W2p", [128, NQH * C], BF16, kind="ExternalInput")
    wab_t = nc.dram_tensor(
        "WABp", [128, NCC * (H + W)], BF16, kind="ExternalInput"
    )
    ba_t = nc.dram_tensor("bAbf", [H], BF16, kind="ExternalInput")
    bb_t = nc.dram_tensor("bBbf", [W], BF16, kind="ExternalInput")
    # [h, (b w)] layout; host swaps to (b, h, w)
    out_t = nc.dram_tensor("out", [H, BS * W], F32, kind="ExternalOutput")
    dbg = {}
    if debug_taps:
        for name, shape in [
            ("dbg_ysum", [128, 33]), ("dbg_ysum_bf", [128, 32]),
            ("dbg_hT", [128, 2 * 2 * BS]), ("dbg_yp", [BS, C]),
            ("dbg_ypT", [128, NCC * BS]), ("dbg_ab", [BS, H + W]),
            ("dbg_bdiag", [BS, BS * W]),
        ]:
            dbg[name] = nc.dram_tensor(name, shape, F32, kind="ExternalOutput")

    # x row r = b*C + cc*128 + p; chunk-major view [cc, p, b, m]
    x_r = x_t[:, :].rearrange("(b q p) m -> q p b m", b=BS, q=NCC, p=128)

    # ---- SBUF ----
    x_sb = nc.alloc_sbuf_tensor("x_sb", [128, NSLOT, BS, HW], F32)
    ysum_sb = nc.alloc_sbuf_tensor("ysum_sb", [128, 33], F32)  # col32 scratch
    ysum_bf = nc.alloc_sbuf_tensor("ysum_bf", [128, 32], BF16)
    w1_sb = nc.alloc_sbuf_tensor("w1_sb", [128, NCC, HID], BF16)
    w2_sb = nc.alloc_sbuf_tensor("w2_sb", [128, NQH, C], BF16)
    wab_sb = nc.alloc_sbuf_tensor("wab_sb", [128, NCC, H + W], BF16)
    bab_sb = nc.alloc_sbuf_tensor("bab_sb", [1, H + W], BF16)
    ident_sb = nc.alloc_sbuf_tensor("ident_sb", [128, 128], BF16)
    ones_sb = nc.alloc_sbuf_tensor("ones_sb", [1, BS], BF16)
    mask_sb = nc.alloc_sbuf_tensor("mask_sb", [BS, BS, W], BF16)
    # hT layout [p, parity, gg, b]: hid group g = 2*gg + parity, so the
    # even/odd-bank gelus each write one contiguous slice
    hT_sb = nc.alloc_sbuf_tensor("hT_sb", [128, 2, 2, BS], BF16)
    yp_sb = nc.alloc_sbuf_tensor("yp_sb", [BS, C], BF16)
    ypT_sb = nc.alloc_sbuf_tensor("ypT_sb", [128, NCC, BS], BF16)
    ab_sb = nc.alloc_sbuf_tensor("ab_sb", [BS, H + W], BF16)
    bdiag_sb = nc.alloc_sbuf_tensor("bdiag_sb", [BS, BS, W], BF16)
    attn_sb = nc.alloc_sbuf_tensor("attn_sb", [H, BS, W], F32)
    scr_sb = nc.alloc_sbuf_tensor("scr_sb", [1, 1], F32)
    # dump target for the ACT-side reduces (activation must write a full
    # output even when only accum_out is wanted)
    red_scr = nc.alloc_sbuf_tensor("red_scr", [128, HW], BF16)

    # ---- PSUM (8 banks) ----
    # hT accumulators: even groups (0,2) / odd groups (1,3) in separate
    # banks so a gelu read never overlaps a PE write to the same bank.
    ps_hte = nc.alloc_psum_tensor("ps_hte", [128, 2, BS], F32)
    ps_hto = nc.alloc_psum_tensor("ps_hto", [128, 2, BS], F32)
    ps_yp1 = nc.alloc_psum_tensor("ps_yp1", [BS, C // 2], F32)
    ps_yp2 = nc.alloc_psum_tensor("ps_yp2", [BS, C // 2], F32)
    tp_a = nc.alloc_psum_tensor("tp_a", [128, 2, BS], BF16)
    tp_b = nc.alloc_psum_tensor("tp_b", [128, 2, BS], BF16)
    ps_ab = nc.alloc_psum_tensor("ps_ab", [BS, H + W], F32)
    ps_at = nc.alloc_psum_tensor("ps_at", [H, BS, W], F32)

    # ---- semaphores ----
    xdma_sems = [nc.alloc_semaphore(f"xdma_sem{n}") for n in range(NT)]
    w_sems = [nc.alloc_semaphore(f"w_sem{i}") for i in range(5)]
    id_sem = nc.alloc_semaphore("id_sem")
    ones_sem = nc.alloc_semaphore("ones_sem")
    red_e = nc.alloc_semaphore("red_e")        # DVE reduce progress
    red_o = nc.alloc_semaphore("red_o")        # ACT reduce progress
    add_sem = nc.alloc_semaphore("add_sem")    # last half-block folded in
    cast_sem = nc.alloc_semaphore("cast_sem")  # +1 per chunk cast (ACT)
    pe_sem = nc.alloc_semaphore("pe_sem")
    act_sem = nc.alloc_semaphore("act_sem")
    dve_sem = nc.alloc_semaphore("dve_sem")
    out_sem = nc.alloc_semaphore("out_sem")

    def red_wait(eng, m):
        """Wait until tile m's reduce is complete on both engines."""
        eng.wait_ge(red_e, RE[m])
        eng.wait_ge(red_o, RO[m])

    # PE ticks (pe_sem after every real PE op):
    #   1..28  mm1T chunks 0..6 (4 pairs each)
    #   29..32 mm1T chunk 7, groups g=0..3
    #   33..36 mm2 half0 q=0..3     37..40 mm2 half1 q=0..3
    #   41..44 tr0..tr3   45,46 m0,m1   47,48 tr4,tr5   49,50 m2,m3
    #   51,52 tr6,tr7   53,54 m4,m5   55,56 m6,m7   57 bias   58 outer
    # ACT increments (act_sem): gelu_hT_even 1, gelu_hT_odd 2,
    #   gelu_yp1 3, gelu_yp2 4, sigmoid 5
    # DVE increments (dve_sem): ypT copies 1..4, ab copy 5, bdiag 6

    with nc.Block() as blk:

        @blk.sync
        def _(sync):
            for n, (cc, b0, b1, m0, m1, sl, sb, gate) in enumerate(TILES):
                if gate >= 0:
                    red_wait(sync, gate)
                sync.dma_start(
                    out=x_sb[:, sl, sb : sb + b1 - b0, 0 : m1 - m0],
                    in_=x_r[cc, :, b0:b1, m0:m1],
                ).then_inc(xdma_sems[n], 16)
            sync.wait_ge(act_sem, 5)
            sync.dma_start(
                out=out_t[:, :],
                in_=attn_sb[:, :, :].rearrange("h b w -> h (b w)"),
            ).then_inc(out_sem, 16)
            sync.wait_ge(out_sem, 16)

        @blk.vector
        def _(vec):
            vec.memset(ones_sb[:, :], 1.0).then_inc(ones_sem, 1)
            for n, (cc, b0, b1, m0, m1, sl, sb, gate) in enumerate(TILES):
                nb = b1 - b0
                if m1 - m0 < HW:
                    if m0 == 0:
                        continue          # T9 is ACT's
                    vec.wait_ge(xdma_sems[n], 16)
                    vec.reduce_sum(       # T10 second half -> scratch col
                        out=ysum_sb[:, 32:33],
                        in_=x_sb[:, sl, sb : sb + 1, 0 : m1 - m0],
                        axis=mybir.AxisListType.X,
                    ).then_inc(red_e, 1)
                    continue
                ne = (nb + 1) // 2        # DVE's share: first ne blocks
                vec.wait_ge(xdma_sems[n], 16)
                vec.reduce_sum(
                    out=ysum_sb[:, cc * BS + b0 : cc * BS + b0 + ne],
                    in_=x_sb[:, sl, sb : sb + ne, :],
                    axis=mybir.AxisListType.X,
                ).then_inc(red_e, 1)
            # fold the halves: col31 (ACT, T9) + col32 (own T10).
            # self-wait: same-engine RAW through SBUF is not interlocked
            vec.wait_ge(red_e, RE[NT - 1])
            vec.wait_ge(red_o, RO[NT - 1])
            nc.vector.tensor_add(
                out=ysum_sb[:, 31:32],
                in0=ysum_sb[:, 31:32],
                in1=ysum_sb[:, 32:33],
            ).then_inc(add_sem, 1)
            # ypT copies: tp_a{tr0,tr1}, tp_b{tr2,tr3}, tp_a{tr4,tr5}, ...
            for i, (bank, tick) in enumerate(
                [(tp_a, 42), (tp_b, 44), (tp_a, 48), (tp_b, 52)]
            ):
                vec.wait_ge(pe_sem, tick)
                nc.vector.tensor_copy(
                    out=ypT_sb[:, 2 * i : 2 * i + 2, :], in_=bank[:, :, :]
                ).then_inc(dve_sem, 1)
            vec.wait_ge(pe_sem, 57)
            nc.vector.tensor_copy(
                out=ab_sb[:, :], in_=ps_ab[:, :]
            ).then_inc(dve_sem, 1)
            # self-wait: the bdiag mul reads ab_sb the copy just wrote
            vec.wait_ge(dve_sem, 5)
            vec.wait_ge(id_sem, 4)
            # bdiag[b, bb, w] = Bv[b, w] * (b == bb)
            b_sl = ab_sb[:, H : H + W]
            b_bc = bass.AP(
                tensor=b_sl.tensor, offset=b_sl.offset,
                ap=[b_sl.ap[0], [0, BS], [b_sl.ap[1][0], W]],
            )
            nc.vector.tensor_mul(
                out=bdiag_sb[:, :, :], in0=b_bc, in1=mask_sb[:, :, :]
            ).then_inc(dve_sem, 1)

        @blk.gpsimd
        def _(gpsimd):
            gpsimd.memset(ident_sb[:, :], 0.0).then_inc(id_sem, 1)
            gpsimd.memset(mask_sb[:, :, :], 0.0).then_inc(id_sem, 1)
            gpsimd.wait_ge(id_sem, 2)
            gpsimd.affine_select(
                out=ident_sb[:, :],
                in_=ident_sb[:, :],
                compare_op=mybir.AluOpType.not_equal,
                fill=1.0,
                base=0,
                pattern=[[-1, 128]],
                channel_multiplier=1,
            ).then_inc(id_sem, 1)
            gpsimd.affine_select(
                out=mask_sb[:, :, :],
                in_=mask_sb[:, :, :],
                compare_op=mybir.AluOpType.not_equal,
                fill=1.0,
                base=0,
                pattern=[[-1, BS], [0, W]],
                channel_multiplier=1,
            ).then_inc(id_sem, 1)
            # all weight loads on the GpSimd SWDGE ring: HWDGE-ring weight
            # DMAs correlate with a persistent ~20% throttle of SDMA
            # engine 15, which starves every x-tile completion semaphore
            gpsimd.dma_start(
                out=w1_sb[:, :, :],
                in_=w1_t[:, :].rearrange("p (n h) -> p n h", n=NCC),
            ).then_inc(w_sems[0], 16)
            gpsimd.dma_start(
                out=bab_sb[0:1, 0:H], in_=ba_t[None, :]
            ).then_inc(w_sems[3], 16)
            gpsimd.dma_start(
                out=bab_sb[0:1, H : H + W], in_=bb_t[None, :]
            ).then_inc(w_sems[4], 16)
            # W2/WAB gated to the stream tail (SWDGE+HWDGE interleave is
            # lossy, so load them as the stream winds down -- they are
            # only needed at the epilogue)
            gpsimd.wait_ge(xdma_sems[7], 16)
            gpsimd.dma_start(
                out=w2_sb[:, :, :],
                in_=w2_t[:, :].rearrange("p (n h) -> p n h", n=NQH),
            ).then_inc(w_sems[1], 16)
            gpsimd.dma_start(
                out=wab_sb[:, :, :],
                in_=wab_t[:, :].rearrange("p (n h) -> p n h", n=NCC),
            ).then_inc(w_sems[2], 16)
            if debug_taps:
                gpsimd.wait_ge(act_sem, 5)
                taps = [
                    (dbg["dbg_ysum"], ysum_sb[:, :]),
                    (dbg["dbg_ysum_bf"], ysum_bf[:, :]),
                    (dbg["dbg_hT"],
                     hT_sb[:, :, :, :].rearrange("p x g b -> p (x g b)")),
                    (dbg["dbg_yp"], yp_sb[:, :]),
                    (dbg["dbg_ypT"],
                     ypT_sb[:, :, :].rearrange("p k b -> p (k b)")),
                    (dbg["dbg_ab"], ab_sb[:, :]),
                    (dbg["dbg_bdiag"],
                     bdiag_sb[:, :, :].rearrange("b bb w -> b (bb w)")),
                ]
                dbg_sem = nc.alloc_semaphore("dbg_sem")
                for i, (dst, src_ap) in enumerate(taps):
                    gpsimd.dma_start(out=dst[:, :], in_=src_ap).then_inc(
                        dbg_sem, 16
                    )
                    gpsimd.wait_ge(dbg_sem, 16 * (i + 1))

        @blk.tensor
        def _(pe):
            pe.wait_ge(w_sems[0], 16)
            # mm1T chunks 0..6 during the stream: hT[g] += W1[cc,g]^T @ ysumT
            for cc in range(7):
                pe.wait_ge(cast_sem, cc + 1)
                for g in range(NQH):
                    bank = ps_hte if g % 2 == 0 else ps_hto
                    nc.tensor.matmul(
                        bank[:, g // 2, :],
                        w1_sb[:, cc, g * 128 : (g + 1) * 128],
                        ysum_bf[:, cc * BS : (cc + 1) * BS],
                        start=(cc == 0),
                        stop=False,
                    ).then_inc(pe_sem, 1)
            # PE clock warm (HAM): the burst must run back-to-back INTO
            # the epilogue -- an idle window in between lets the clock
            # ramp decay again (measured: mm2 at ~0.8GHz vs 1.36GHz)
            pe.wait_ge(id_sem, 4)
            pe.wait_ge(ones_sem, 1)
            red_wait(pe, 7)
            for _i in range(20):
                nc.tensor.matmul(
                    ps_ab[:, :], ident_sb[:, 0:BS], ident_sb[:, 0 : H + W],
                    start=True, stop=True,
                )
            pe.wait_ge(xdma_sems[NT - 1], 16)
            for _i in range(8):
                nc.tensor.matmul(
                    ps_ab[:, :], ident_sb[:, 0:BS], ident_sb[:, 0 : H + W],
                    start=True, stop=True,
                )
            # mm1T chunk 7 (ticks 29..32)
            pe.wait_ge(cast_sem, 8)
            for g in range(NQH):
                bank = ps_hte if g % 2 == 0 else ps_hto
                nc.tensor.matmul(
                    bank[:, g // 2, :],
                    w1_sb[:, 7, g * 128 : (g + 1) * 128],
                    ysum_bf[:, 7 * BS : 8 * BS],
                    start=False,
                    stop=True,
                ).then_inc(pe_sem, 1)
            # mm2: yp[b, :] = hT^T @ W2   (ticks 33..40)
            pe.wait_ge(w_sems[1], 16)
            for half in range(2):
                for q in range(NQH):
                    if half == 0 and q < 2:
                        pe.wait_ge(act_sem, q + 1)
                    nc.tensor.matmul(
                        (ps_yp1 if half == 0 else ps_yp2)[:, :],
                        hT_sb[:, q % 2, q // 2, :],
                        w2_sb[:, q, half * 512 : (half + 1) * 512],
                        start=(q == 0),
                        stop=(q == NQH - 1),
                    ).then_inc(pe_sem, 1)
            # yp transposes + mm3 interleaved (ticks 41..57)
            pe.wait_ge(w_sems[2], 16)
            tr_banks = [tp_a, tp_a, tp_b, tp_b, tp_a, tp_a, tp_b, tp_b]

            def tr(k):
                if k == 0:
                    pe.wait_ge(act_sem, 3)
                if k == 4:
                    pe.wait_ge(act_sem, 4)
                    pe.wait_ge(dve_sem, 1)   # tp_a drained
                if k == 6:
                    pe.wait_ge(dve_sem, 2)   # tp_b drained
                nc.tensor.transpose(
                    tr_banks[k][:, k % 2, :],
                    yp_sb[:, k * 128 : (k + 1) * 128],
                    ident_sb[:BS, :BS],
                ).then_inc(pe_sem, 1)

            def m3(k, copy_idx):
                pe.wait_ge(dve_sem, copy_idx)
                nc.tensor.matmul(
                    ps_ab[:, :],
                    ypT_sb[:, k, :],
                    wab_sb[:, k, :],
                    start=(k == 0),
                    stop=False,
                ).then_inc(pe_sem, 1)

            tr(0); tr(1); tr(2); tr(3)
            m3(0, 1); m3(1, 1)
            tr(4); tr(5)
            m3(2, 2); m3(3, 2)
            tr(6); tr(7)
            m3(4, 3); m3(5, 3)
            m3(6, 4); m3(7, 4)
            pe.wait_ge(w_sems[3], 16)
            pe.wait_ge(w_sems[4], 16)
            nc.tensor.matmul(
                ps_ab[:, :], ones_sb[:, :], bab_sb[:, :],
                start=False, stop=True,
            ).then_inc(pe_sem, 1)
            # outer product vs block-diag Bv (tick 58)
            pe.wait_ge(dve_sem, 6)
            nc.tensor.matmul(
                ps_at[:, :, :].rearrange("h b w -> h (b w)"),
                ab_sb[:, 0:H],
                bdiag_sb[:, :, :].rearrange("b bb w -> b (bb w)"),
                start=True, stop=True,
            ).then_inc(pe_sem, 1)

        @blk.scalar
        def _(act):
            # dummy gelu loads the ACT table early, off the critical path
            zero = nc.const_aps.aps[(F32, 0.0)]
            nc.scalar.activation(scr_sb[0:1, :], zero[0:1, :], gelu_fn)
            # ACT-side reduces (activation accum_out = free-axis row sum)
            # interleaved with the per-chunk ysum casts f32 -> bf16
            for n, (cc, b0, b1, m0, m1, sl, sb, gate) in enumerate(TILES):
                nb = b1 - b0
                if m1 - m0 < HW:
                    if m0 != 0:
                        continue          # T10 is DVE's
                    act.wait_ge(xdma_sems[n], 16)
                    nc.scalar.activation(   # T9 first half -> col 31
                        red_scr[:, 0 : m1 - m0],
                        x_sb[:, sl, sb, 0 : m1 - m0],
                        copy_fn,
                        accum_out=ysum_sb[:, 31:32],
                    ).then_inc(red_o, 1)
                    continue
                ne = (nb + 1) // 2
                act.wait_ge(xdma_sems[n], 16)
                for j in range(ne, nb):   # ACT: blocks ne..nb-1, one each
                    nc.scalar.activation(
                        red_scr[:, :],
                        x_sb[:, sl, sb + j, :],
                        copy_fn,
                        accum_out=ysum_sb[:, cc * BS + b0 + j : cc * BS
                                          + b0 + j + 1],
                    ).then_inc(red_o, 1)
                if cc < 7 and b1 == 4:
                    # cast cc: self-wait drains own reduce writes (RAW)
                    act.wait_ge(red_o, RO[n])
                    act.wait_ge(red_e, RE[n])
                    nc.scalar.copy(
                        out=ysum_bf[:, cc * BS : (cc + 1) * BS],
                        in_=ysum_sb[:, cc * BS : (cc + 1) * BS],
                    ).then_inc(cast_sem, 1)
            act.wait_ge(red_o, RO[NT - 1])
            act.wait_ge(add_sem, 1)
            nc.scalar.copy(
                out=ysum_bf[:, 28:32], in_=ysum_sb[:, 28:32]
            ).then_inc(cast_sem, 1)
            # gelu on h^T straight out of PSUM (scale folds in the 1/HW)
            act.wait_ge(pe_sem, 31)
            nc.scalar.activation(
                hT_sb[:, 0, :, :], ps_hte[:, :, :], gelu_fn, scale=1.0 / HW
            ).then_inc(act_sem, 1)
            act.wait_ge(pe_sem, 32)
            nc.scalar.activation(
                hT_sb[:, 1, :, :], ps_hto[:, :, :], gelu_fn, scale=1.0 / HW
            ).then_inc(act_sem, 1)
            act.wait_ge(pe_sem, 36)
            nc.scalar.activation(
                yp_sb[:, 0 : C // 2], ps_yp1[:, :], gelu_fn
            ).then_inc(act_sem, 1)
            act.wait_ge(pe_sem, 40)
            nc.scalar.activation(
                yp_sb[:, C // 2 : C], ps_yp2[:, :], gelu_fn
            ).then_inc(act_sem, 1)
            # dummy sigmoid swaps the ACT table while PE transposes run
            nc.scalar.activation(scr_sb[0:1, :], zero[0:1, :], sig_fn)
            act.wait_ge(pe_sem, 58)
            nc.scalar.activation(
                attn_sb[:, :, :], ps_at[:, :, :], sig_fn
            ).then_inc(act_sem, 1)

    return nc


_NC_CACHE: list = []


def run_on_hw(x, W1, W2, WA, bA, WB, bB, **spmd_kwargs):
    """Run the SPMD kernel; returns (full_output, BassKernelResults)."""
    import ml_dtypes

    bf = ml_dtypes.bfloat16
    x = np.ascontiguousarray(np.asarray(x, dtype=np.float32))
    # pack weights into SBUF layout: [p, n, ...] where row n*128+p -> (p, n)
    w1p = np.asarray(W1).reshape(NCC, 128, HID).transpose(1, 0, 2)
    w2p = np.asarray(W2).reshape(NQH, 128, C).transpose(1, 0, 2)
    wab = np.concatenate([np.asarray(WA), np.asarray(WB)], axis=1)  # (C, H+W)
    wabp = wab.reshape(NCC, 128, H + W).transpose(1, 0, 2)
    weights = {
        "W1p": np.ascontiguousarray(w1p.reshape(128, NCC * HID).astype(bf)),
        "W2p": np.ascontiguousarray(w2p.reshape(128, NQH * C).astype(bf)),
        "WABp": np.ascontiguousarray(
            wabp.reshape(128, NCC * (H + W)).astype(bf)
        ),
        "bAbf": np.ascontiguousarray(np.asarray(bA).astype(bf)),
        "bBbf": np.ascontiguousarray(np.asarray(bB).astype(bf)),
    }

    if not _NC_CACHE:
        _NC_CACHE.append(build_bass())
    nc = _NC_CACHE[0]

    in_maps = []
    for i in range(NCORES):
        shard = x[i * BS : (i + 1) * BS].reshape(ROWS, HW)
        in_maps.append({"x": shard, **weights})

    res = run_bass_kernel_spmd(
        nc, in_maps, core_ids=list(range(NCORES)), **spmd_kwargs
    )
    # per-core out is [H, BS*W]; swap to (BS, H, W) then concat cores
    attn = np.concatenate(
        [r["out"].reshape(H, BS, W).transpose(1, 0, 2) for r in res.results],
        axis=0,
    )  # (B, H, W)
    return np.broadcast_to(attn.reshape(B, 1, H, W), (B, C, H, W)), res


def kernel(x, W1, W2, WA, bA, WB, bB):
    out, _ = run_on_hw(x, W1, W2, WA, bA, WB, bB)
    return out


# revision 50
# speedup vs baseline: 1.2240x; 1.0496x over previous
"""Trainium2 Bass kernel for nn_AdaptiveBlock (dense_mlp).

Reference computation:
    y    = mean(x, axis=(2, 3))                   # (B, C) global avg pool
    h    = gelu(y @ W1)                           # (B, HID), exact erf gelu
    yp   = gelu(h @ W2)                           # (B, C)
    A    = yp @ WA + bA                           # (B, H)
    Bv   = yp @ WB + bB                           # (B, W)
    attn = sigmoid(A[:,None,:,None] * Bv[:,None,None,:])   # (B, 1, H, W)
    out  = broadcast(attn, (B, C, H, W))

Sharding: data-parallel over batch across 8 NeuronCores (4 batches/core),
weights replicated, no collectives.  Each core streams its 51.4 MB x-shard
through SBUF, row-reduces it, runs the tiny MLP on-chip, and writes only
its (56, 4*56) attention map.  The channel broadcast (and the h<->b axis
swap) is done on the host; it carries no information.

Key design points (measured on HW, see git history of this file):
  * x streams CHUNK-major: each 4-block tile completes one contraction
    chunk of mm1, so mm1 runs DURING the stream.  mm1 is computed
    TRANSPOSED (lhsT = W1 chunk [128c x 128hid], rhs = ysumT [128c x 4b]),
    which also yields h^T directly -- no PE transposes of h before mm2.
  * The x DMA issue is gated to <=3 outstanding tiles.  A deep HWDGE ring
    makes SDMA engine 15 run ~20% slow (starving every tile's completion
    semaphore); a shallow ring keeps the engines balanced.
  * Each tile's reduce is split DVE (blocks 0-1) / ACT accum_out (blocks
    2-3), halving per-tile reduce latency so the DMA gate always clears
    early and the stream never stalls.
  * Weights ride the SCALAR HWDGE ring (second hardware ring, full rate,
    no SWDGE interleave penalty); host pre-packs them into SBUF layout so
    each weight DMA is one big contiguous descriptor per partition.
  * Tail tiles shrink to 2/1/half blocks so the last reduce is ~1.7us.
  * Same-engine back-to-back RAW through SBUF is NOT interlocked (relaxed
    ordering): every such hazard carries an explicit self-wait.
  * Single sigmoid, single contiguous 50KB output DMA ([H, BS*W] layout).
"""

import numpy as np

import concourse.bass as bass
from concourse import mybir
from concourse.bass_utils import run_bass_kernel_spmd

B, C, HID, H, W = 32, 1024, 512, 56, 56
NCORES = 8
BS = B // NCORES          # 4 batches per core
ROWS = BS * C             # 4096 (b, c) rows per core
HW = H * W                # 3136
NCC = C // 128            # 8 contraction chunks for mm1
NQH = HID // 128          # 4 contraction chunks for mm2
NSLOT = 3                 # x slots (4 blocks each)
F32 = mybir.dt.float32
BF16 = mybir.dt.bfloat16

# x tiles, chunk-major: (cc, b_lo, b_hi, m_lo, m_hi, slot, sblk, gate).
# T0..T6 = chunks 0-6 (4 blocks); chunk 7 tapers: 2 + 1 + half + half.
# The small chunk-7 tiles pack into free blocks of slots whose previous
# tile is long reduced (gate = index of that previous tile), so the tail
# DMAs issue back-to-back and never wait on the reduce chain.
TILES = [(cc, 0, 4, 0, HW, cc % 3, 0, cc - 3) for cc in range(7)] + [
    (7, 0, 2, 0, HW, 1, 0, 4),        # T7: slot1 blocks 0-1 (after T4)
    (7, 2, 3, 0, HW, 1, 2, 4),        # T8: slot1 block 2
    (7, 3, 4, 0, HW // 2, 2, 0, 5),   # T9: slot2 block 0 (after T5)
    (7, 3, 4, HW // 2, HW, 2, 1, 5),  # T10: slot2 block 1
]
NT = len(TILES)  # 11

# per-tile reduce split: DVE takes the first ceil(nb/2) blocks, ACT the
# rest -- except the half-block tiles T9 (ACT) / T10 (DVE).
# cumulative (red_e, red_o) counts after tile m's reduce is done:
RE, RO = [], []
e = o = 0
for m, (cc, b0, b1, m0, m1, _sl, _sb, _g) in enumerate(TILES):
    nb = b1 - b0
    if m1 - m0 < HW:          # half tiles: T9 on ACT, T10 on DVE
        if m0 == 0:
            o += 1
        else:
            e += 1
    else:
        e += 1                # DVE: one reduce over its block span
        o += nb - (nb + 1) // 2   # ACT: one accum per block
    RE.append(e)
    RO.append(o)


def build_bass(gelu_fn=None, debug_taps=False) -> bass.Bass:
    if gelu_fn is None:
        gelu_fn = mybir.ActivationFunctionType.Gelu
    sig_fn = mybir.ActivationFunctionType.Sigmoid
    copy_fn = mybir.ActivationFunctionType.Copy
    nc = bass.Bass()

    x_t = nc.dram_tensor("x", [ROWS, HW], F32, kind="ExternalInput")
    w1_t = nc.dram_tensor("W1p", [128, NCC * HID], BF16, kind="ExternalInput")
    w2_t = nc.dram_tensor("W2p", [128, NQH * C], BF16, kind="ExternalInput")
    wab_t = nc.dram_tensor(
        "WABp", [128, NCC * (H + W)], BF16, kind="ExternalInput"
    )
    ba_t = nc.dram_tensor("bAbf", [H], BF16, kind="ExternalInput")
    bb_t = nc.dram_tensor("bBbf", [W], BF16, kind="ExternalInput")
    # [h, (b w)] layout; host swaps to (b, h, w)
    out_t = nc.dram_tensor("out", [H, BS * W], F32, kind="ExternalOutput")
    dbg = {}
    if debug_taps:
        for name, shape in [
            ("dbg_ysum", [128, 33]), ("dbg_ysum_bf", [128, 32]),
            ("dbg_hT", [128, 2 * 2 * BS]), ("dbg_yp", [BS, C]),
            ("dbg_ypT", [128, NCC * BS]), ("dbg_ab", [BS, H + W]),
            ("dbg_bdiag", [BS, BS * W]),
        ]:
            dbg[name] = nc.dram_tensor(name, shape, F32, kind="ExternalOutput")

    # x row r = b*C + cc*128 + p; chunk-major view [cc, p, b, m]
    x_r = x_t[:, :].rearrange("(b q p) m -> q p b m", b=BS, q=NCC, p=128)

    # ---- SBUF ----
    x_sb = nc.alloc_sbuf_tensor("x_sb", [128, NSLOT, BS, HW], F32)
    ysum_sb = nc.alloc_sbuf_tensor("ysum_sb", [128, 33], F32)  # col32 scratch
    ysum_bf = nc.alloc_sbuf_tensor("ysum_bf", [128, 32], BF16)
    w1_sb = nc.alloc_sbuf_tensor("w1_sb", [128, NCC, HID], BF16)
    w2_sb = nc.alloc_sbuf_tensor("w2_sb", [128, NQH, C], BF16)
    wab_sb = nc.alloc_sbuf_tensor("wab_sb", [128, NCC, H + W], BF16)
    bab_sb = nc.alloc_sbuf_tensor("bab_sb", [1, H + W], BF16)
    ident_sb = nc.alloc_sbuf_tensor("ident_sb", [128, 128], BF16)
    ones_sb = nc.alloc_sbuf_tensor("ones_sb", [1, BS], BF16)
    mask_sb = nc.alloc_sbuf_tensor("mask_sb", [BS, BS, W], BF16)
    # hT layout [p, parity, gg, b]: hid group g = 2*gg + parity, so the
    # even/odd-bank gelus each write one contiguous slice
    hT_sb = nc.alloc_sbuf_tensor("hT_sb", [128, 2, 2, BS], BF16)
    yp_sb = nc.alloc_sbuf_tensor("yp_sb", [BS, C], BF16)
    ypT_sb = nc.alloc_sbuf_tensor("ypT_sb", [128, NCC, BS], BF16)
    ab_sb = nc.alloc_sbuf_tensor("ab_sb", [BS, H + W], BF16)
    bdiag_sb = nc.alloc_sbuf_tensor("bdiag_sb", [BS, BS, W], BF16)
    attn_sb = nc.alloc_sbuf_tensor("attn_sb", [H, BS, W], F32)
    scr_sb = nc.alloc_sbuf_tensor("scr_sb", [1, 1], F32)
    # dump target for the ACT-side reduces (activation must write a full
    # output even when only accum_out is wanted)
    red_scr = nc.alloc_sbuf_tensor("red_scr", [128, HW], BF16)

    # ---- PSUM (8 banks) ----
    # hT accumulators: even groups (0,2) / odd groups (1,3) in separate
    # banks so a gelu read never overlaps a PE write to the same bank.
    ps_hte = nc.alloc_psum_tensor("ps_hte", [128, 2, BS], F32)
    ps_hto = nc.alloc_psum_tensor("ps_hto", [128, 2, BS], F32)
    ps_yp1 = nc.alloc_psum_tensor("ps_yp1", [BS, C // 2], F32)
    ps_yp2 = nc.alloc_psum_tensor("ps_yp2", [BS, C // 2], F32)
    tp_a = nc.alloc_psum_tensor("tp_a", [128, 2, BS], BF16)
    tp_b = nc.alloc_psum_tensor("tp_b", [128, 2, BS], BF16)
    ps_ab = nc.alloc_psum_tensor("ps_ab", [BS, H + W], F32)
    ps_at = nc.alloc_psum_tensor("ps_at", [H, BS, W], F32)

    # ---- semaphores ----
    xdma_sems = [nc.alloc_semaphore(f"xdma_sem{n}") for n in range(NT)]
    w_sems = [nc.alloc_semaphore(f"w_sem{i}") for i in range(5)]
    id_sem = nc.alloc_semaphore("id_sem")
    ones_sem = nc.alloc_semaphore("ones_sem")
    red_e = nc.alloc_semaphore("red_e")        # DVE reduce progress
    red_o = nc.alloc_semaphore("red_o")        # ACT reduce progress
    add_sem = nc.alloc_semaphore("add_sem")    # last half-block folded in
    cast_sem = nc.alloc_semaphore("cast_sem")  # +1 per chunk cast (ACT)
    pe_sem = nc.alloc_semaphore("pe_sem")
    act_sem = nc.alloc_semaphore("act_sem")
    dve_sem = nc.alloc_semaphore("dve_sem")
    out_sem = nc.alloc_semaphore("out_sem")

    def red_wait(eng, m):
        """Wait until tile m's reduce is complete on both engines."""
        eng.wait_ge(red_e, RE[m])
        eng.wait_ge(red_o, RO[m])

    # PE ticks (pe_sem after every real PE op):
    #   1..28  mm1T chunks 0..6 (4 pairs each)
    #   29..32 mm1T chunk 7, groups g=0..3
    #   33..36 mm2 half0 q=0..3     37..40 mm2 half1 q=0..3
    #   41..44 tr0..tr3   45,46 m0,m1   47,48 tr4,tr5   49,50 m2,m3
    #   51,52 tr6,tr7   53,54 m4,m5   55,56 m6,m7   57 bias   58 outer
    # ACT increments (act_sem): gelu_hT_even 1, gelu_hT_odd 2,
    #   gelu_yp1 3, gelu_yp2 4, sigmoid 5
    # DVE increments (dve_sem): ypT copies 1..4, ab copy 5, bdiag 6

    with nc.Block() as blk:

        @blk.sync
        def _(sync):
            for n, (cc, b0, b1, m0, m1, sl, sb, gate) in enumerate(TILES):
                if gate >= 0:
                    red_wait(sync, gate)
                sync.dma_start(
                    out=x_sb[:, sl, sb : sb + b1 - b0, 0 : m1 - m0],
                    in_=x_r[cc, :, b0:b1, m0:m1],
                ).then_inc(xdma_sems[n], 16)
            # two output halves: the first DMA's HBM write receipt
            # overlaps the second half's sigmoid + transfer
            sync.wait_ge(act_sem, 5)
            sync.dma_start(
                out=out_t[0:32, :],
                in_=attn_sb[0:32, :, :].rearrange("h b w -> h (b w)"),
            ).then_inc(out_sem, 16)
            sync.wait_ge(act_sem, 6)
            sync.dma_start(
                out=out_t[32:H, :],
                in_=attn_sb[32:H, :, :].rearrange("h b w -> h (b w)"),
            ).then_inc(out_sem, 16)
            sync.wait_ge(out_sem, 32)

        @blk.vector
        def _(vec):
            vec.memset(ones_sb[:, :], 1.0).then_inc(ones_sem, 1)
            for n, (cc, b0, b1, m0, m1, sl, sb, gate) in enumerate(TILES):
                nb = b1 - b0
                if m1 - m0 < HW:
                    if m0 == 0:
                        continue          # T9 is ACT's
                    vec.wait_ge(xdma_sems[n], 16)
                    vec.reduce_sum(       # T10 second half -> scratch col
                        out=ysum_sb[:, 32:33],
                        in_=x_sb[:, sl, sb : sb + 1, 0 : m1 - m0],
                        axis=mybir.AxisListType.X,
                    ).then_inc(red_e, 1)
                    continue
                ne = (nb + 1) // 2        # DVE's share: first ne blocks
                vec.wait_ge(xdma_sems[n], 16)
                vec.reduce_sum(
                    out=ysum_sb[:, cc * BS + b0 : cc * BS + b0 + ne],
                    in_=x_sb[:, sl, sb : sb + ne, :],
                    axis=mybir.AxisListType.X,
                ).then_inc(red_e, 1)
            # fold the halves: col31 (ACT, T9) + col32 (own T10).
            # self-wait: same-engine RAW through SBUF is not interlocked
            vec.wait_ge(red_e, RE[NT - 1])
            vec.wait_ge(red_o, RO[NT - 1])
            nc.vector.tensor_add(
                out=ysum_sb[:, 31:32],
                in0=ysum_sb[:, 31:32],
                in1=ysum_sb[:, 32:33],
            ).then_inc(add_sem, 1)
            # ypT copies: tp_a{tr0,tr1}, tp_b{tr2,tr3}, tp_a{tr4,tr5}, ...
            for i, (bank, tick) in enumerate(
                [(tp_a, 42), (tp_b, 44), (tp_a, 48), (tp_b, 52)]
            ):
                vec.wait_ge(pe_sem, tick)
                nc.vector.tensor_copy(
                    out=ypT_sb[:, 2 * i : 2 * i + 2, :], in_=bank[:, :, :]
                ).then_inc(dve_sem, 1)
            vec.wait_ge(pe_sem, 57)
            nc.vector.tensor_copy(
                out=ab_sb[:, :], in_=ps_ab[:, :]
            ).then_inc(dve_sem, 1)
            # self-wait: the bdiag mul reads ab_sb the copy just wrote
            vec.wait_ge(dve_sem, 5)
            vec.wait_ge(id_sem, 4)
            # bdiag[b, bb, w] = Bv[b, w] * (b == bb)
            b_sl = ab_sb[:, H : H + W]
            b_bc = bass.AP(
                tensor=b_sl.tensor, offset=b_sl.offset,
                ap=[b_sl.ap[0], [0, BS], [b_sl.ap[1][0], W]],
            )
            nc.vector.tensor_mul(
                out=bdiag_sb[:, :, :], in0=b_bc, in1=mask_sb[:, :, :]
            ).then_inc(dve_sem, 1)

        @blk.gpsimd
        def _(gpsimd):
            gpsimd.memset(ident_sb[:, :], 0.0).then_inc(id_sem, 1)
            gpsimd.memset(mask_sb[:, :, :], 0.0).then_inc(id_sem, 1)
            gpsimd.wait_ge(id_sem, 2)
            gpsimd.affine_select(
                out=ident_sb[:, :],
                in_=ident_sb[:, :],
                compare_op=mybir.AluOpType.not_equal,
                fill=1.0,
                base=0,
                pattern=[[-1, 128]],
                channel_multiplier=1,
            ).then_inc(id_sem, 1)
            gpsimd.affine_select(
                out=mask_sb[:, :, :],
                in_=mask_sb[:, :, :],
                compare_op=mybir.AluOpType.not_equal,
                fill=1.0,
                base=0,
                pattern=[[-1, BS], [0, W]],
                channel_multiplier=1,
            ).then_inc(id_sem, 1)
            # all weight loads on the GpSimd SWDGE ring: HWDGE-ring weight
            # DMAs correlate with a persistent ~20% throttle of SDMA
            # engine 15, which starves every x-tile completion semaphore
            gpsimd.dma_start(
                out=w1_sb[:, :, :],
                in_=w1_t[:, :].rearrange("p (n h) -> p n h", n=NCC),
            ).then_inc(w_sems[0], 16)
            gpsimd.dma_start(
                out=bab_sb[0:1, 0:H], in_=ba_t[None, :]
            ).then_inc(w_sems[3], 16)
            gpsimd.dma_start(
                out=bab_sb[0:1, H : H + W], in_=bb_t[None, :]
            ).then_inc(w_sems[4], 16)
            # W2/WAB gated to the stream tail (SWDGE+HWDGE interleave is
            # lossy, so load them as the stream winds down -- they are
            # only needed at the epilogue)
            gpsimd.wait_ge(xdma_sems[9], 16)
            gpsimd.dma_start(
                out=w2_sb[:, :, :],
                in_=w2_t[:, :].rearrange("p (n h) -> p n h", n=NQH),
            ).then_inc(w_sems[1], 16)
            gpsimd.dma_start(
                out=wab_sb[:, :, :],
                in_=wab_t[:, :].rearrange("p (n h) -> p n h", n=NCC),
            ).then_inc(w_sems[2], 16)
            if debug_taps:
                gpsimd.wait_ge(act_sem, 5)
                taps = [
                    (dbg["dbg_ysum"], ysum_sb[:, :]),
                    (dbg["dbg_ysum_bf"], ysum_bf[:, :]),
                    (dbg["dbg_hT"],
                     hT_sb[:, :, :, :].rearrange("p x g b -> p (x g b)")),
                    (dbg["dbg_yp"], yp_sb[:, :]),
                    (dbg["dbg_ypT"],
                     ypT_sb[:, :, :].rearrange("p k b -> p (k b)")),
                    (dbg["dbg_ab"], ab_sb[:, :]),
                    (dbg["dbg_bdiag"],
                     bdiag_sb[:, :, :].rearrange("b bb w -> b (bb w)")),
                ]
                dbg_sem = nc.alloc_semaphore("dbg_sem")
                for i, (dst, src_ap) in enumerate(taps):
                    gpsimd.dma_start(out=dst[:, :], in_=src_ap).then_inc(
                        dbg_sem, 16
                    )
                    gpsimd.wait_ge(dbg_sem, 16 * (i + 1))

        @blk.tensor
        def _(pe):
            pe.wait_ge(w_sems[0], 16)
            # mm1T chunks 0..6 during the stream: hT[g] += W1[cc,g]^T @ ysumT
            for cc in range(7):
                pe.wait_ge(cast_sem, cc + 1)
                for g in range(NQH):
                    bank = ps_hte if g % 2 == 0 else ps_hto
                    nc.tensor.matmul(
                        bank[:, g // 2, :],
                        w1_sb[:, cc, g * 128 : (g + 1) * 128],
                        ysum_bf[:, cc * BS : (cc + 1) * BS],
                        start=(cc == 0),
                        stop=False,
                    ).then_inc(pe_sem, 1)
            # PE clock warm (HAM): the burst must run back-to-back INTO
            # the epilogue -- an idle window in between lets the clock
            # ramp decay again (measured: mm2 at ~0.8GHz vs 1.36GHz)
            pe.wait_ge(id_sem, 4)
            pe.wait_ge(ones_sem, 1)
            red_wait(pe, 7)
            for _i in range(64):
                nc.tensor.matmul(
                    ps_ab[:, :], ident_sb[:, 0:BS], ident_sb[:, 0 : H + W],
                    start=True, stop=True,
                )
            pe.wait_ge(xdma_sems[NT - 1], 16)
            for _i in range(16):
                nc.tensor.matmul(
                    ps_ab[:, :], ident_sb[:, 0:BS], ident_sb[:, 0 : H + W],
                    start=True, stop=True,
                )
            # mm1T chunk 7 (ticks 29..32)
            pe.wait_ge(cast_sem, 8)
            for g in range(NQH):
                bank = ps_hte if g % 2 == 0 else ps_hto
                nc.tensor.matmul(
                    bank[:, g // 2, :],
                    w1_sb[:, 7, g * 128 : (g + 1) * 128],
                    ysum_bf[:, 7 * BS : 8 * BS],
                    start=False,
                    stop=True,
                ).then_inc(pe_sem, 1)
            # mm2: yp[b, :] = hT^T @ W2   (ticks 33..40)
            pe.wait_ge(w_sems[1], 16)
            for half in range(2):
                for q in range(NQH):
                    if half == 0 and q < 2:
                        pe.wait_ge(act_sem, q + 1)
                    nc.tensor.matmul(
                        (ps_yp1 if half == 0 else ps_yp2)[:, :],
                        hT_sb[:, q % 2, q // 2, :],
                        w2_sb[:, q, half * 512 : (half + 1) * 512],
                        start=(q == 0),
                        stop=(q == NQH - 1),
                    ).then_inc(pe_sem, 1)
            # yp transposes + mm3 interleaved (ticks 41..57)
            pe.wait_ge(w_sems[2], 16)
            tr_banks = [tp_a, tp_a, tp_b, tp_b, tp_a, tp_a, tp_b, tp_b]

            def tr(k):
                if k == 0:
                    pe.wait_ge(act_sem, 3)
                if k == 4:
                    pe.wait_ge(act_sem, 4)
                    pe.wait_ge(dve_sem, 1)   # tp_a drained
                if k == 6:
                    pe.wait_ge(dve_sem, 2)   # tp_b drained
                nc.tensor.transpose(
                    tr_banks[k][:, k % 2, :],
                    yp_sb[:, k * 128 : (k + 1) * 128],
                    ident_sb[:BS, :BS],
                ).then_inc(pe_sem, 1)

            def m3(k, copy_idx):
                pe.wait_ge(dve_sem, copy_idx)
                nc.tensor.matmul(
                    ps_ab[:, :],
                    ypT_sb[:, k, :],
                    wab_sb[:, k, :],
                    start=(k == 0),
                    stop=False,
                ).then_inc(pe_sem, 1)

            tr(0); tr(1); tr(2); tr(3)
            m3(0, 1); m3(1, 1)
            tr(4); tr(5)
            m3(2, 2); m3(3, 2)
            tr(6); tr(7)
            m3(4, 3); m3(5, 3)
            m3(6, 4); m3(7, 4)
            pe.wait_ge(w_sems[3], 16)
            pe.wait_ge(w_sems[4], 16)
            nc.tensor.matmul(
                ps_ab[:, :], ones_sb[:, :], bab_sb[:, :],
                start=False, stop=True,
            ).then_inc(pe_sem, 1)
            # outer product vs block-diag Bv (tick 58)
            pe.wait_ge(dve_sem, 6)
            nc.tensor.matmul(
                ps_at[:, :, :].rearrange("h b w -> h (b w)"),
                ab_sb[:, 0:H],
                bdiag_sb[:, :, :].rearrange("b bb w -> b (bb w)"),
                start=True, stop=True,
            ).then_inc(pe_sem, 1)

        @blk.scalar
        def _(act):
            # dummy gelu loads the ACT table early, off the critical path
            zero = nc.const_aps.aps[(F32, 0.0)]
            nc.scalar.activation(scr_sb[0:1, :], zero[0:1, :], gelu_fn)
            # ACT-side reduces (activation accum_out = free-axis row sum)
            # interleaved with the per-chunk ysum casts f32 -> bf16
            for n, (cc, b0, b1, m0, m1, sl, sb, gate) in enumerate(TILES):
                nb = b1 - b0
                if m1 - m0 < HW:
                    if m0 != 0:
                        continue          # T10 is DVE's
                    act.wait_ge(xdma_sems[n], 16)
                    nc.scalar.activation(   # T9 first half -> col 31
                        red_scr[:, 0 : m1 - m0],
                        x_sb[:, sl, sb, 0 : m1 - m0],
                        copy_fn,
                        accum_out=ysum_sb[:, 31:32],
                    ).then_inc(red_o, 1)
                    continue
                ne = (nb + 1) // 2
                act.wait_ge(xdma_sems[n], 16)
                for j in range(ne, nb):   # ACT: blocks ne..nb-1, one each
                    nc.scalar.activation(
                        red_scr[:, :],
                        x_sb[:, sl, sb + j, :],
                        copy_fn,
                        accum_out=ysum_sb[:, cc * BS + b0 + j : cc * BS
                                          + b0 + j + 1],
                    ).then_inc(red_o, 1)
                if cc < 7 and b1 == 4:
                    # cast cc: self-wait drains own reduce writes (RAW)
                    act.wait_ge(red_o, RO[n])
                    act.wait_ge(red_e, RE[n])
                    nc.scalar.copy(
                        out=ysum_bf[:, cc * BS : (cc + 1) * BS],
                        in_=ysum_sb[:, cc * BS : (cc + 1) * BS],
                    ).then_inc(cast_sem, 1)
            act.wait_ge(red_o, RO[NT - 1])
            act.wait_ge(add_sem, 1)
            nc.scalar.copy(
                out=ysum_bf[:, 28:32], in_=ysum_sb[:, 28:32]
            ).then_inc(cast_sem, 1)
            # gelu on h^T straight out of PSUM (scale folds in the 1/HW)
            act.wait_ge(pe_sem, 31)
            nc.scalar.activation(
                hT_sb[:, 0, :, :], ps_hte[:, :, :], gelu_fn, scale=1.0 / HW
            ).then_inc(act_sem, 1)
            act.wait_ge(pe_sem, 32)
            nc.scalar.activation(
                hT_sb[:, 1, :, :], ps_hto[:, :, :], gelu_fn, scale=1.0 / HW
            ).then_inc(act_sem, 1)
            act.wait_ge(pe_sem, 36)
            nc.scalar.activation(
                yp_sb[:, 0 : C // 2], ps_yp1[:, :], gelu_fn
            ).then_inc(act_sem, 1)
            act.wait_ge(pe_sem, 40)
            nc.scalar.activation(
                yp_sb[:, C // 2 : C], ps_yp2[:, :], gelu_fn
            ).then_inc(act_sem, 1)
            # dummy sigmoid swaps the ACT table while PE transposes run
            nc.scalar.activation(scr_sb[0:1, :], zero[0:1, :], sig_fn)
            act.wait_ge(pe_sem, 58)
            nc.scalar.activation(
                attn_sb[0:32, :, :], ps_at[0:32, :, :], sig_fn
            ).then_inc(act_sem, 1)
            nc.scalar.activation(
                attn_sb[32:H, :, :], ps_at[32:H, :, :], sig_fn
            ).then_inc(act_sem, 1)

    return nc


_NC_CACHE: list = []


def run_on_hw(x, W1, W2, WA, bA, WB, bB, **spmd_kwargs):
    """Run the SPMD kernel; returns (full_output, BassKernelResults)."""
    import ml_dtypes

    bf = ml_dtypes.bfloat16
    x = np.ascontiguousarray(np.asarray(x, dtype=np.float32))
    # pack weights into SBUF layout: [p, n, ...] where row n*128+p -> (p, n)
    w1p = np.asarray(W1).reshape(NCC, 128, HID).transpose(1, 0, 2)
    w2p = np.asarray(W2).reshape(NQH, 128, C).transpose(1, 0, 2)
    wab = np.concatenate([np.asarray(WA), np.asarray(WB)], axis=1)  # (C, H+W)
    wabp = wab.reshape(NCC, 128, H + W).transpose(1, 0, 2)
    weights = {
        "W1p": np.ascontiguousarray(w1p.reshape(128, NCC * HID).astype(bf)),
        "W2p": np.ascontiguousarray(w2p.reshape(128, NQH * C).astype(bf)),
        "WABp": np.ascontiguousarray(
            wabp.reshape(128, NCC * (H + W)).astype(bf)
        ),
        "bAbf": np.ascontiguousarray(np.asarray(bA).astype(bf)),
        "bBbf": np.ascontiguousarray(np.asarray(bB).astype(bf)),
    }

    if not _NC_CACHE:
        _NC_CACHE.append(build_bass())
    nc = _NC_CACHE[0]

    in_maps = []
    for i in range(NCORES):
        shard = x[i * BS : (i + 1) * BS].reshape(ROWS, HW)
        in_maps.append({"x": shard, **weights})

    res = run_bass_kernel_spmd(
        nc, in_maps, core_ids=list(range(NCORES)), **spmd_kwargs
    )
    # per-core out is [H, BS*W]; swap to (BS, H, W) then concat cores
    attn = np.concatenate(
        [r["out"].reshape(H, BS, W).transpose(1, 0, 2) for r in res.results],
        axis=0,
    )  # (B, H, W)
    return np.broadcast_to(attn.reshape(B, 1, H, W), (B, C, H, W)), res


def kernel(x, W1, W2, WA, bA, WB, bB):
    out, _ = run_on_hw(x, W1, W2, WA, bA, WB, bB)
    return out
